# revision 1
# baseline (speedup 1.0000x reference)
"""Trainium2 Bass kernel for nn_Decoder (LSTM-style decoder with r/dt side path).

Reference math (per step t, teacher forcing):
    xs_t    = SOS one-hot (t=0) or input_seq[:, t-1]
    z       = xs_t @ w2h_w.T + w2h_b + hid @ h2h_w.T + h2h_b          (B, 4H)
    gi,gf,go = sigmoid(z[:, 0:H]), sigmoid(z[:, H:2H]), sigmoid(z[:, 2H:3H])
    chat    = tanh(z[:, 3H:4H])
    gr      = sigmoid(xs_t @ w2h_r_w.T + w2h_r_b + a*(hid @ h2h_r_w.T + h2h_r_b))
    dt      = gr * dt
    cell    = gf*cell + gi*chat + dt @ dc_w.T
    hid     = go * tanh(cell)
    logits  = hid @ out_w.T + out_b

Distribution: tensor-parallel over H across 8 cores (128 H-dims per core).
  - GEMM1 (xs @ w2h.T): each core computes its 512 gate rows for all 4800
    (t,b) columns; r-projection is column-split 600/core + one AllGather.
  - scan: per-core 640-dim gate slice, per-step 16KB AllGather of hidT chunks.
  - GEMM2 (logits): vocab-split 1000/core.
All layouts are transposed: feature dims on SBUF partitions, (t,b) on free.
"""

import functools

import numpy as np
import ml_dtypes

B = 64
T = 75
V = 8000
H = 1024
D = 128
ALPHA = 0.5
NCORE = 8
HC = H // NCORE          # 128: per-core hidden chunk
GS = 4 * HC              # 512: per-core gate rows
VS = V // NCORE          # 1000: per-core vocab slice
TB = T * B               # 4800
V_PAD = 8064             # 63 * 128
KV = V_PAD // 128        # 63 K-tiles for GEMM1
KH = H // 128            # 8 K-tiles for the scan / GEMM2
NCH = 200                # GEMM1 n-chunk columns (divides TB and TB/NCORE)
NCHUNKS = TB // NCH      # 24
RCH = (TB // NCORE) // NCH   # 5 r-chunks per core

BF16 = ml_dtypes.bfloat16
DEBUG_TAPS = False
# True = issue all GEMM1 units before the scan; False = interleave GEMM1
# units between scan steps so they fill the AllGather wait gaps (~0.5ms
# faster). Both validated on hardware at full size.
G1_PROLOGUE_ALL = False
NO_COLLECTIVES = False  # timing-bisect only: replaces AGs with local DMAs (WRONG results)
# Exchange hidden-state chunks with direct SBUF->SBUF remote DMA broadcasts
# (XOR slot addressing) instead of ncfw AllGather collectives (~15us each).
REMOTE_EXCHANGE = False


def _build_module(t_steps=T, v_pad=V_PAD, nch=NCH, vs=VS):
    import concourse.mybir as mybir
    import concourse.tile as tile
    from concourse import bacc

    dt_ = mybir.dt
    f32, bf16 = dt_.float32, dt_.bfloat16
    AF = mybir.ActivationFunctionType
    ALU = mybir.AluOpType

    kv = v_pad // 128
    tb = t_steps * B
    nchunks = tb // nch
    rch = (tb // NCORE) // nch
    RG = [list(range(NCORE))]
    # GEMM2 column split into <=500-wide pieces (PSUM bank limit)
    g2_splits = []
    col = 0
    while col < vs:
        w = min(500, vs - col)
        g2_splits.append((col, w))
        col += w

    nc = bacc.Bacc("TRN2", target_bir_lowering=False, num_devices=NCORE)

    # ---------------- I/O ----------------
    xs_ch = nc.dram_tensor("xs_ch", [nchunks, 128, kv, nch], bf16, kind="ExternalInput")
    xs_r = nc.dram_tensor("xs_r", [rch, 128, kv, nch], bf16, kind="ExternalInput")
    w1T = nc.dram_tensor("w1T", [v_pad, GS + D], bf16, kind="ExternalInput")
    wcatT = nc.dram_tensor("wcatT", [H, 5 * HC], bf16, kind="ExternalInput")
    dcT = nc.dram_tensor("dcT", [D, HC], bf16, kind="ExternalInput")
    owT = nc.dram_tensor("owT", [H, vs], bf16, kind="ExternalInput")
    obB = nc.dram_tensor("obB", [128, vs], f32, kind="ExternalInput")
    biasC = nc.dram_tensor("biasC", [128, 5], f32, kind="ExternalInput")
    identI = nc.dram_tensor("identI", [128, 128], bf16, kind="ExternalInput")
    hidT0 = nc.dram_tensor("hidT0", [H, B], bf16, kind="ExternalInput")
    cellT0 = nc.dram_tensor("cellT0", [HC, B], f32, kind="ExternalInput")
    dtT0 = nc.dram_tensor("dtT0", [D, B], f32, kind="ExternalInput")
    outc = nc.dram_tensor("outc", [tb, vs], f32, kind="ExternalOutput")
    if DEBUG_TAPS:
        dbg_pre = nc.dram_tensor("dbg_pre", [5 * HC, tb], bf16, kind="ExternalOutput")
        dbg_hid = nc.dram_tensor("dbg_hid", [t_steps, H, B], bf16, kind="ExternalOutput")

    if REMOTE_EXCHANGE:
        # hidden-state history for GEMM2, staged per step from SBUF
        hidst = nc.dram_tensor("hidst", [t_steps, 128, KH, B], bf16)
        recv_sem = nc.alloc_semaphore("rdma_recv")
        prep_sem = nc.alloc_semaphore("rdma_prep")
        lsem = nc.alloc_semaphore("rdma_local")
    else:
        # per-step AllGather buffers (must persist until GEMM2)
        agi = [nc.dram_tensor(f"agi{t}", [128, B], bf16) for t in range(t_steps)]
        ago = [
            nc.dram_tensor(f"ago{t}", [H, B], bf16, addr_space="Shared")
            for t in range(t_steps)
        ]
    agr_i = nc.dram_tensor("agr_i", [128, tb // NCORE], bf16)
    agr_o = nc.dram_tensor("agr_o", [H, tb // NCORE], bf16, addr_space="Shared")

    with tile.TileContext(nc) as tc:
        import contextlib

        with contextlib.ExitStack() as ctx:
            cpool = ctx.enter_context(tc.tile_pool(name="const", bufs=1))
            spool = ctx.enter_context(tc.tile_pool(name="state", bufs=1))
            wpool = ctx.enter_context(tc.tile_pool(name="work", bufs=3))
            hpool = ctx.enter_context(tc.tile_pool(name="hid", bufs=3))
            # ---- resident constants ----
            wcat_sb = cpool.tile([128, KH, 5 * HC], bf16)
            nc.sync.dma_start(wcat_sb[:], wcatT.ap().rearrange("(k p) m -> p k m", p=128))
            dc_sb = cpool.tile([128, HC], bf16)
            nc.sync.dma_start(dc_sb[:], dcT.ap())
            bias_sb = cpool.tile([128, 5], f32)
            nc.sync.dma_start(bias_sb[:], biasC.ap())
            id_sb = cpool.tile([128, 128], bf16)
            nc.sync.dma_start(id_sb[:], identI.ap())
            preG = cpool.tile([128, 4, tb], bf16)
            preR = cpool.tile([128, tb], bf16)

            # ---- state ----
            cell_sb = spool.tile([128, B], f32)
            nc.sync.dma_start(cell_sb[:], cellT0.ap())
            dt_sb = spool.tile([128, B], f32)
            nc.sync.dma_start(dt_sb[:], dtT0.ap())

            if REMOTE_EXCHANGE:
                hstA = spool.tile([128, KH, B], bf16, name="hstA")
                hstB = spool.tile([128, KH, B], bf16, name="hstB")
                hcur = hstA
                nc.sync.dma_start(
                    hcur[:], hidT0.ap().rearrange("(k p) n -> p k n", p=128)
                )
            else:
                hcur = hpool.tile([128, KH, B], bf16, tag="hstage")
                nc.sync.dma_start(
                    hcur[:], hidT0.ap().rearrange("(k p) n -> p k n", p=128)
                )

            with contextlib.ExitStack() as c1:
                g1pool = c1.enter_context(tc.tile_pool(name="g1", bufs=1))
                xpool = c1.enter_context(tc.tile_pool(name="xs", bufs=2))
                gpsum = c1.enter_context(
                    tc.tile_pool(name="gpsum", bufs=2, space="PSUM")
                )
                zpsum = c1.enter_context(
                    tc.tile_pool(name="zpsum", bufs=1, space="PSUM")
                )
                dpsum = c1.enter_context(
                    tc.tile_pool(name="dpsum", bufs=1, space="PSUM")
                )

                w1_sb = g1pool.tile([128, kv, GS + D], bf16)
                nc.sync.dma_start(
                    w1_sb[:], w1T.ap().rearrange("(k p) m -> p k m", p=128)
                )

                # ---- prologue: r-projection (column slice) + AllGather ----
                prr = g1pool.tile([128, tb // NCORE], bf16)
                for i in range(rch):
                    xt = xpool.tile([128, kv, nch], bf16, tag="xsch")
                    nc.sync.dma_start(xt[:], xs_r.ap()[i])
                    pg = gpsum.tile([128, nch], f32, tag="gps")
                    for k in range(kv):
                        nc.tensor.matmul(
                            pg[:],
                            w1_sb[:, k, GS : GS + D],
                            xt[:, k, :],
                            start=(k == 0),
                            stop=(k == kv - 1),
                        )
                    nc.vector.tensor_scalar_add(
                        prr[:, i * nch : (i + 1) * nch], pg[:], bias_sb[:, 0:1]
                    )
                nc.sync.dma_start(agr_i.ap(), prr[:])
                if NO_COLLECTIVES:
                    for j in range(NCORE):
                        nc.sync.dma_start(
                            agr_o.ap()[j * 128 : (j + 1) * 128], agr_i.ap()
                        )
                    for t in range(t_steps):
                        nc.sync.dma_start(ago[t].ap(), hidT0.ap())
                else:
                    nc.gpsimd.collective_compute(
                        "AllGather",
                        ALU.bypass,
                        replica_groups=RG,
                        ins=[agr_i.ap().opt()],
                        outs=[agr_o.ap().opt()],
                    )
                nc.sync.dma_start(
                    preR[:].rearrange("p (c f) -> p c f", c=NCORE),
                    agr_o.ap().rearrange("(c p) f -> p c f", p=128),
                )

                # ---- GEMM1 gate units, interleaved with the scan ----
                xs_tiles = {}

                def chunk_dma(ch):
                    if ch in xs_tiles or ch >= nchunks:
                        return
                    xt = xpool.tile([128, kv, nch], bf16, tag="xsch")
                    nc.sync.dma_start(xt[:], xs_ch.ap()[ch])
                    xs_tiles[ch] = xt

                state = {"issued": 0}

                def issue_units(target):
                    while state["issued"] < min(target, 4 * nchunks):
                        u = state["issued"]
                        ch, g = u // 4, u % 4
                        if g == 0:
                            chunk_dma(ch)
                            chunk_dma(ch + 1)
                        pg = gpsum.tile([128, nch], f32, tag="gps")
                        for k in range(kv):
                            nc.tensor.matmul(
                                pg[:],
                                w1_sb[:, k, g * HC : (g + 1) * HC],
                                xs_tiles[ch][:, k, :],
                                start=(k == 0),
                                stop=(k == kv - 1),
                            )
                        nc.vector.tensor_scalar_add(
                            preG[:, g, ch * nch : (ch + 1) * nch],
                            pg[:],
                            bias_sb[:, 1 + g : 2 + g],
                        )
                        state["issued"] += 1
                        if state["issued"] % 4 == 0:
                            xs_tiles.pop(state["issued"] // 4 - 1, None)

                def g1_target(t):
                    if G1_PROLOGUE_ALL:
                        return 4 * nchunks
                    deadline = 4 * (((t + 2) * B) // nch + 1)
                    den = max(1, t_steps - 3)
                    pace = (4 * nchunks * (t + 1) + den - 1) // den
                    return min(4 * nchunks, max(deadline, pace))

                issue_units(g1_target(0))

                # ---- the scan ----
                for t in range(t_steps):
                    # one PSUM tile (= one bank) per gate region: start=True
                    # clears has_written for the WHOLE bank, so accumulation
                    # groups must not share a bank.
                    pz = [
                        zpsum.tile([128, B], f32, tag=f"pz{m}", name=f"pz{m}_{t}")
                        for m in range(5)
                    ]
                    # inject pre-projections (identity matmul, one LDW)
                    for m in range(5):
                        pre_ap = (
                            preR[:, t * B : (t + 1) * B]
                            if m == 0
                            else preG[:, m - 1, t * B : (t + 1) * B]
                        )
                        nc.tensor.matmul(
                            pz[m][:], id_sb[:], pre_ap, start=True, stop=False
                        )
                    # recurrent matmuls
                    for m in range(5):
                        for k in range(KH):
                            nc.tensor.matmul(
                                pz[m][:],
                                wcat_sb[:, k, m * HC : (m + 1) * HC],
                                hcur[:, k, :],
                                start=False,
                                stop=(k == KH - 1),
                            )
                    # activations: [r | gi | gf | go] sigmoid, [chat] tanh
                    sg = wpool.tile([128, 4 * B], f32, tag="sg")
                    for m in range(4):
                        nc.scalar.activation(
                            sg[:, m * B : (m + 1) * B], pz[m][:], AF.Sigmoid
                        )
                    th = wpool.tile([128, B], f32, tag="th")
                    nc.scalar.activation(th[:], pz[4][:], AF.Tanh)
                    # dt = gr * dt ; dc = dcT.T @ dt
                    nc.vector.tensor_mul(dt_sb[:], sg[:, 0:B], dt_sb[:])
                    dtb = wpool.tile([128, B], bf16, tag="dtb")
                    nc.vector.tensor_copy(dtb[:], dt_sb[:])
                    pdc = dpsum.tile([128, B], f32, tag="pdc")
                    nc.tensor.matmul(pdc[:], dc_sb[:], dtb[:], start=True, stop=True)
                    # cell = gf*cell + gi*chat + dc
                    tmp = wpool.tile([128, B], f32, tag="tmp")
                    nc.vector.tensor_mul(tmp[:], sg[:, B : 2 * B], th[:])
                    nc.vector.tensor_mul(cell_sb[:], sg[:, 2 * B : 3 * B], cell_sb[:])
                    nc.vector.tensor_add(cell_sb[:], cell_sb[:], tmp[:])
                    nc.vector.tensor_add(cell_sb[:], cell_sb[:], pdc[:])
                    # hid = go * tanh(cell)
                    thc = wpool.tile([128, B], f32, tag="thc")
                    nc.scalar.activation(thc[:], cell_sb[:], AF.Tanh)
                    hch = wpool.tile([128, B], bf16, tag="hch")
                    nc.vector.tensor_mul(hch[:], sg[:, 3 * B : 4 * B], thc[:])
                    # issue GEMM1 filler work BEFORE the exchange so it can
                    # run on the PE while the exchange is in flight
                    if t + 1 < t_steps:
                        issue_units(g1_target(t + 1))
                    # exchange hidden chunks
                    if REMOTE_EXCHANGE:
                        hnx = hstB if t % 2 == 0 else hstA
                        with tc.tile_critical():
                            g = nc.gpsimd
                            if t >= 1:
                                g.wait_ge(lsem, 128 * t)
                            for k in range(NCORE):
                                g.remote_dma_broadcast(
                                    out_ap=hnx[:, k, :],
                                    in_ap=hch[:],
                                    remote_sem=recv_sem,
                                    local_sem=lsem,
                                    rdests=[
                                        (0, j) if j == k else None
                                        for j in range(NCORE)
                                    ],
                                ).then_inc(prep_sem, 1)
                            g.wait_ge(prep_sem, NCORE * (t + 1))
                            g.trigger_dma(count=NCORE)
                            # arrival fence: sync engine waits for all 16
                            # lane-halves, then self-copies the staging tile
                            # so Tile's dependency tracking gates all its
                            # readers on actual data arrival.
                            nc.sync.wait_ge(recv_sem, 2 * NCORE * (t + 1))
                            nc.sync.dma_start(hnx[:], hnx[:])
                        nc.sync.dma_start(hidst.ap()[t], hnx[:])
                        hcur = hnx
                    else:
                        nc.sync.dma_start(agi[t].ap(), hch[:])
                        if NO_COLLECTIVES:
                            nc.sync.dma_start(ago[t].ap()[0:128], agi[t].ap())
                        else:
                            nc.gpsimd.collective_compute(
                                "AllGather",
                                ALU.bypass,
                                replica_groups=RG,
                                ins=[agi[t].ap().opt()],
                                outs=[ago[t].ap().opt()],
                            )
                        if t + 1 < t_steps:
                            hcur = hpool.tile([128, KH, B], bf16, tag="hstage")
                            nc.sync.dma_start(
                                hcur[:],
                                ago[t].ap().rearrange("(k p) n -> p k n", p=128),
                            )

            if DEBUG_TAPS:
                for t in range(t_steps):
                    dcp = wpool.tile([128, KH, B], bf16, tag="dcp")
                    nc.sync.dma_start(
                        dcp[:], ago[t].ap().rearrange("(k p) n -> p k n", p=128)
                    )
                    nc.sync.dma_start(
                        dbg_hid.ap()[t].rearrange("(k p) n -> p k n", p=128), dcp[:]
                    )
                nc.sync.dma_start(dbg_pre.ap()[0:HC], preR[:])
                for g in range(4):
                    nc.sync.dma_start(
                        dbg_pre.ap()[(1 + g) * HC : (2 + g) * HC], preG[:, g, :]
                    )

            # ---- GEMM2: logits = hidT.T @ owT + ob ----
            with contextlib.ExitStack() as c2:
                g2pool = c2.enter_context(tc.tile_pool(name="g2", bufs=3))
                opsum = c2.enter_context(
                    tc.tile_pool(name="opsum", bufs=2, space="PSUM")
                )
                ow_sb = g2pool.tile([128, KH, vs], bf16, tag="owt", bufs=1)
                nc.sync.dma_start(
                    ow_sb[:], owT.ap().rearrange("(k p) n -> p k n", p=128)
                )
                ob_sb = g2pool.tile([128, vs], f32, tag="obb", bufs=1)
                nc.sync.dma_start(ob_sb[:], obB.ap())

                n_rb = (tb + 127) // 128
                for rb in range(n_rb):
                    rows = min(128, tb - rb * 128)
                    lh = g2pool.tile([128, KH, 128], bf16, tag="g2lh")
                    if REMOTE_EXCHANGE:
                        nc.sync.dma_start(lh[:, :, 0:B], hidst.ap()[2 * rb])
                        if rows > B:
                            nc.sync.dma_start(lh[:, :, B:128], hidst.ap()[2 * rb + 1])
                    else:
                        nc.sync.dma_start(
                            lh[:, :, 0:B],
                            ago[2 * rb].ap().rearrange("(k p) n -> p k n", p=128),
                        )
                        if rows > B:
                            nc.sync.dma_start(
                                lh[:, :, B:128],
                                ago[2 * rb + 1]
                                .ap()
                                .rearrange("(k p) n -> p k n", p=128),
                            )
                    # one PSUM tile per split: a matmul output must not cross
                    # a 2KB bank boundary
                    osb = g2pool.tile([128, vs], f32, tag="osb")
                    for j, (c0, w) in enumerate(g2_splits):
                        po = opsum.tile(
                            [128, w], f32, tag=f"po{j}", name=f"po{j}_{rb}"
                        )
                        for k in range(KH):
                            nc.tensor.matmul(
                                po[:rows],
                                lh[:, k, 0:rows],
                                ow_sb[:, k, c0 : c0 + w],
                                start=(k == 0),
                                stop=(k == KH - 1),
                            )
                        nc.vector.tensor_add(
                            osb[:rows, c0 : c0 + w],
                            po[:rows],
                            ob_sb[:rows, c0 : c0 + w],
                        )
                    nc.sync.dma_start(
                        outc.ap()[rb * 128 : rb * 128 + rows, :], osb[:rows]
                    )

    nc.finalize()
    return nc


@functools.lru_cache(maxsize=2)
def _cached_module(t_steps, v_pad, nch, vs):
    return _build_module(t_steps, v_pad, nch, vs)


def _prep_inputs(
    input_seq, last_hidden, last_dt, w2h_w, w2h_b, h2h_w, h2h_b,
    w2h_r_w, w2h_r_b, h2h_r_w, h2h_r_b, dc_w, out_w, out_b,
):
    """Host-side sharding/layout. Returns per-core input dicts."""
    b, t_steps, v = input_seq.shape
    h = last_hidden.shape[1]
    d = last_dt.shape[1]
    tb = t_steps * b
    v_pad = ((v + 127) // 128) * 128
    kv = v_pad // 128
    # choose n-chunk: must divide tb and tb // NCORE
    nch = NCH if (tb % NCH == 0 and (tb // NCORE) % NCH == 0) else (tb // NCORE)
    while tb % nch or (tb // NCORE) % nch:
        nch //= 2
    nchunks = tb // nch
    rch = (tb // NCORE) // nch
    vs = v // NCORE

    # xsT: (v_pad, tb) with col t*B+b = SOS (t=0) or input_seq[b, t-1]
    xsT = np.zeros((v_pad, tb), np.float32)
    xsT[0, 0:b] = 1.0
    xsT[:v, b:] = input_seq[:, : t_steps - 1, :].transpose(2, 1, 0).reshape(v, tb - b)
    xsT = xsT.astype(BF16)
    # chunked layout (nchunks, 128, kv, nch)
    xs_ch = np.ascontiguousarray(
        xsT.reshape(kv, 128, nchunks, nch).transpose(2, 1, 0, 3)
    )

    gate_bias = (w2h_b + h2h_b).astype(np.float32)
    r_bias = (w2h_r_b + ALPHA * h2h_r_b).astype(np.float32)

    ident = np.eye(128, dtype=BF16)
    hidT0 = np.ascontiguousarray(last_hidden.T).astype(BF16)
    dtT0 = np.ascontiguousarray(last_dt.T).astype(np.float32)
    cellT0_full = np.ascontiguousarray(last_hidden.T).astype(np.float32)

    wrT = np.zeros((v_pad, d), BF16)
    wrT[:v] = w2h_r_w.T.astype(BF16)
    wcat_r = (ALPHA * h2h_r_w).T.astype(BF16)  # (h, d)

    in_maps = []
    for c in range(NCORE):
        idx = np.concatenate(
            [np.arange(g * h + c * HC, g * h + (c + 1) * HC) for g in range(4)]
        )
        # row permutation of the H axis: with REMOTE_EXCHANGE, staging slot j
        # on core c holds H-chunk (c XOR j), so per-core H-contracted weights
        # are supplied with their K-tiles in that order.
        if REMOTE_EXCHANGE:
            hperm = np.concatenate(
                [np.arange((c ^ j) * HC, (c ^ j) * HC + HC) for j in range(NCORE)]
            )
        else:
            hperm = np.arange(h)
        w1 = np.zeros((v_pad, GS + d), BF16)
        w1[:v, :GS] = w2h_w[idx].T.astype(BF16)
        w1[:, GS:] = wrT
        wcat = np.concatenate([wcat_r, h2h_w[idx].T.astype(BF16)], axis=1)[hperm]
        biasC = np.zeros((128, 5), np.float32)
        biasC[:, 0] = r_bias
        for g in range(4):
            biasC[:, 1 + g] = gate_bias[g * h + c * HC : g * h + (c + 1) * HC]
        in_maps.append(
            {
                "xs_ch": xs_ch,
                "xs_r": xs_ch[c * rch : (c + 1) * rch],
                "w1T": w1,
                "wcatT": np.ascontiguousarray(wcat),
                "dcT": np.ascontiguousarray(dc_w[c * HC : (c + 1) * HC, :].T).astype(
                    BF16
                ),
                "owT": np.ascontiguousarray(
                    out_w[c * vs : (c + 1) * vs, :].T[hperm]
                ).astype(BF16),
                "obB": np.ascontiguousarray(
                    np.broadcast_to(
                        out_b[c * vs : (c + 1) * vs].astype(np.float32), (128, vs)
                    )
                ),
                "biasC": biasC,
                "identI": ident,
                "hidT0": np.ascontiguousarray(hidT0[hperm]),
                "cellT0": np.ascontiguousarray(cellT0_full[c * HC : (c + 1) * HC]),
                "dtT0": dtT0,
            }
        )
    return in_maps, nch, v_pad, vs


def kernel(**inputs):
    from concourse.bass_utils import run_bass_kernel_spmd

    input_seq = np.asarray(inputs["input_seq"], np.float32)
    b, t_steps, v = input_seq.shape
    args = {
        k: np.asarray(inputs[k], np.float32)
        for k in (
            "last_hidden", "last_dt", "w2h_w", "w2h_b", "h2h_w", "h2h_b",
            "w2h_r_w", "w2h_r_b", "h2h_r_w", "h2h_r_b", "dc_w", "out_w", "out_b",
        )
    }
    in_maps, nch, v_pad, vs = _prep_inputs(input_seq, **{
        "last_hidden": args["last_hidden"], "last_dt": args["last_dt"],
        "w2h_w": args["w2h_w"], "w2h_b": args["w2h_b"],
        "h2h_w": args["h2h_w"], "h2h_b": args["h2h_b"],
        "w2h_r_w": args["w2h_r_w"], "w2h_r_b": args["w2h_r_b"],
        "h2h_r_w": args["h2h_r_w"], "h2h_r_b": args["h2h_r_b"],
        "dc_w": args["dc_w"], "out_w": args["out_w"], "out_b": args["out_b"],
    })
    nc = _cached_module(t_steps, v_pad, nch, vs)
    res = run_bass_kernel_spmd(nc, in_maps, core_ids=list(range(NCORE)))
    stack = np.stack([res.results[c]["outc"] for c in range(NCORE)])  # (8, tb, vs)
    out = (
        stack.reshape(NCORE, t_steps, b, vs)
        .transpose(2, 1, 0, 3)
        .reshape(b, t_steps, NCORE * vs)
    )
    return np.ascontiguousarray(out)



# revision 6
# speedup vs baseline: 2.3611x; 2.3611x over previous
"""Trainium2 Bass kernel for nn_Decoder (LSTM-style decoder with r/dt side path).

Reference math (per step t, teacher forcing):
    xs_t    = SOS one-hot (t=0) or input_seq[:, t-1]
    z       = xs_t @ w2h_w.T + w2h_b + hid @ h2h_w.T + h2h_b          (B, 4H)
    gi,gf,go = sigmoid(z[:, 0:H]), sigmoid(z[:, H:2H]), sigmoid(z[:, 2H:3H])
    chat    = tanh(z[:, 3H:4H])
    gr      = sigmoid(xs_t @ w2h_r_w.T + w2h_r_b + a*(hid @ h2h_r_w.T + h2h_r_b))
    dt      = gr * dt
    cell    = gf*cell + gi*chat + dt @ dc_w.T
    hid     = go * tanh(cell)
    logits  = hid @ out_w.T + out_b

Distribution: data-parallel over batch (the sharding_hint's primary option).
Each of the 8 cores runs 8 of the 64 sequences end-to-end with replicated
weights — no collectives and no cross-core synchronization anywhere.

Per-core schedule (features on partitions, (t, b_local) on free dims):
  GEMM1  pre = w1.T @ xs   (4224, 600): 33 output row-tiles x 2 column
         halves; w1 (66MB) is streamed through SBUF one row-tile at a time
         (twice, once per column half) so xs half + w1 tile + pre all fit.
  scan   75 steps. All 41 per-step PSUM slices (33 z-tiles + 8 dc-tiles)
         live in ONE 2KB PSUM bank: a single identity-matmul injection
         (start=True) pending-zeroes the bank and seeds z with pre[t];
         every following matmul accumulates with start=False into its own
         disjoint slice. One sigmoid activation covers r|gi|gf|go (200
         cols), one tanh covers chat. The hidden state history is written
         straight into a resident SBUF tile that both the next step's
         matmuls and GEMM2 read — no DRAM round-trip in the scan.
  GEMM2  logits = ow.T @ hist  (8000, 600): vocab streamed in 8 chunks.
"""

import functools

import numpy as np
import ml_dtypes

B = 64
T = 75
V = 8000
H = 1024
D = 128
ALPHA = 0.5
NCORE = 8
BL = B // NCORE          # 8: per-core batch
COLS = T * BL            # 600: per-core (t, b) columns
V_PAD = 8064             # 63 * 128
KV = V_PAD // 128        # 63 K-tiles for GEMM1
KH = H // 128            # 8 K-tiles for the scan / GEMM2
NM = 33                  # GEMM1 / z output row tiles: r(1) + gates(32)
NZ = NM + KH             # 41: z tiles + dc tiles share one PSUM bank
NVT = V_PAD // 128       # 63 vocab tiles
NCH = COLS               # kept for test.py compatibility

BF16 = ml_dtypes.bfloat16
# GEMM2 vocab chunks (tile_start, n_tiles)
G2_CHUNKS = [(i, min(8, NVT - i)) for i in range(0, NVT, 8)]


def _build_module(t_steps=T, v_pad=V_PAD, nch=NCH, vs=V):
    import concourse.mybir as mybir
    import concourse.tile as tile
    from concourse import bacc

    dt_ = mybir.dt
    f32, bf16 = dt_.float32, dt_.bfloat16
    AF = mybir.ActivationFunctionType

    cols = t_steps * BL

    nc = bacc.Bacc("TRN2", target_bir_lowering=False, num_devices=NCORE)

    # ---------------- I/O ----------------
    xsT = nc.dram_tensor("xsT", [v_pad, cols], bf16, kind="ExternalInput")
    w1ch = nc.dram_tensor("w1ch", [NM, 128, KV, 128], bf16, kind="ExternalInput")
    wcatT = nc.dram_tensor("wcatT", [H, NM * 128], bf16, kind="ExternalInput")
    dcT = nc.dram_tensor("dcT", [D, H], bf16, kind="ExternalInput")
    owT = nc.dram_tensor("owT", [H, v_pad], bf16, kind="ExternalInput")
    biasG = nc.dram_tensor("biasG", [128, NM], f32, kind="ExternalInput")
    biasO = nc.dram_tensor("biasO", [128, NVT], f32, kind="ExternalInput")
    identI = nc.dram_tensor("identI", [128, 128], bf16, kind="ExternalInput")
    hidT0 = nc.dram_tensor("hidT0", [H, BL], bf16, kind="ExternalInput")
    cellT0 = nc.dram_tensor("cellT0", [H, BL], f32, kind="ExternalInput")
    dtT0 = nc.dram_tensor("dtT0", [D, BL], f32, kind="ExternalInput")
    outc = nc.dram_tensor("outc", [NVT, 128, cols], f32, kind="ExternalOutput")

    with tile.TileContext(nc) as tc:
        import contextlib

        with contextlib.ExitStack() as ctx:
            cpool = ctx.enter_context(tc.tile_pool(name="const", bufs=1))
            spool = ctx.enter_context(tc.tile_pool(name="state", bufs=1))

            # resident constants / accumulators
            pre = cpool.tile([128, t_steps, NM, BL], bf16)       # 38.7KB/part
            dc_sb = cpool.tile([128, H], bf16)
            nc.sync.dma_start(dc_sb[:], dcT.ap())
            bg_sb = cpool.tile([128, NM], f32)
            nc.sync.dma_start(bg_sb[:], biasG.ap())
            bo_sb = cpool.tile([128, NVT], f32)
            nc.sync.dma_start(bo_sb[:], biasO.ap())
            id_sb = cpool.tile([128, 128], bf16)
            nc.sync.dma_start(id_sb[:], identI.ap())

            # state
            hid0_sb = spool.tile([128, KH, BL], bf16)
            nc.sync.dma_start(
                hid0_sb[:], hidT0.ap().rearrange("(k p) n -> p k n", p=128)
            )
            cell_sb = spool.tile([128, KH, BL], f32)
            nc.sync.dma_start(
                cell_sb[:], cellT0.ap().rearrange("(k p) n -> p k n", p=128)
            )
            dt_sb = spool.tile([128, BL], f32)
            nc.sync.dma_start(dt_sb[:], dtT0.ap())

            # ---- GEMM1: pre[m-tile, (t,b)] = w1[:, m].T @ xs + bias ----
            # xs stays resident; w1 (66MB) streams through one row-tile at a
            # time in a single pass. Column split into two PSUM chunks (the
            # 600 f32 columns exceed one 2KB bank).
            with contextlib.ExitStack() as c1:
                xpool = c1.enter_context(tc.tile_pool(name="xs", bufs=1))
                w1pool = c1.enter_context(tc.tile_pool(name="w1", bufs=2))
                gpsum = c1.enter_context(
                    tc.tile_pool(name="g1p", bufs=2, space="PSUM")
                )
                xs_sb = xpool.tile([128, KV, cols], bf16, tag="xs")
                nc.sync.dma_start(
                    xs_sb[:], xsT.ap().rearrange("(k p) n -> p k n", p=128)
                )
                for m in range(NM):
                    w1sb = w1pool.tile([128, KV, 128], bf16, tag="w1")
                    nc.sync.dma_start(w1sb[:], w1ch.ap()[m])
                    for h0, hw in ((0, 304), (304, 296)):
                        pg = gpsum.tile(
                            [128, 304], f32, tag="pg", name=f"pg{h0}_{m}"
                        )
                        for k in range(KV):
                            nc.tensor.matmul(
                                pg[:, 0:hw],
                                w1sb[:, k, :],
                                xs_sb[:, k, h0 : h0 + hw],
                                start=(k == 0),
                                stop=(k == KV - 1),
                            )
                        nc.vector.tensor_scalar_add(
                            pre[:, h0 // BL : (h0 + hw) // BL, m, :],
                            pg[:, 0:hw],
                            bg_sb[:, m : m + 1],
                        )

            # ---- the scan: 75 fully-local steps ----
            # wcat + the hidden history live in a pool that opens after the
            # GEMM1 pools free their SBUF, and stays open through GEMM2.
            with contextlib.ExitStack() as cs:
                hpool = cs.enter_context(tc.tile_pool(name="hp", bufs=1))
                wcat_sb = hpool.tile([128, KH, NM * 128], bf16)  # 66KB/part
                nc.sync.dma_start(
                    wcat_sb[:], wcatT.ap().rearrange("(k p) m -> p k m", p=128)
                )
                hist = hpool.tile([128, KH, cols], bf16)  # hidden history
                wpool = cs.enter_context(tc.tile_pool(name="work", bufs=2))
                zpool = cs.enter_context(
                    tc.tile_pool(name="zp", bufs=2, space="PSUM")
                )
                for t in range(t_steps):
                    # one bank: [r|gi|gf|go|chat](33) + dc(8), all f32 x BL
                    pz = zpool.tile([128, NZ, BL], f32, tag="z", name=f"z{t}")
                    # identity injection seeds z with pre[t] and
                    # pending-zeroes the whole bank (incl. the dc slices)
                    nc.tensor.matmul(
                        pz[:, 0:NM, :], id_sb[:], pre[:, t, :, :],
                        start=True, stop=False,
                    )
                    for k in range(KH):
                        rhs = (
                            hid0_sb[:, k, :]
                            if t == 0
                            else hist[:, k, (t - 1) * BL : t * BL]
                        )
                        for m in range(NM):
                            nc.tensor.matmul(
                                pz[:, m, :],
                                wcat_sb[:, k, m * 128 : (m + 1) * 128],
                                rhs,
                                start=False,
                                stop=False,
                            )
                    # activations: [r|gi|gf|go] sigmoid in one shot, chat tanh
                    sg = wpool.tile([128, 25, BL], f32, tag="sg")
                    nc.scalar.activation(sg[:], pz[:, 0:25, :], AF.Sigmoid)
                    th = wpool.tile([128, KH, BL], f32, tag="th")
                    nc.scalar.activation(th[:], pz[:, 25:NM, :], AF.Tanh)
                    # dt = gr * dt ; dc = dc_w @ dt accumulated into the bank
                    nc.vector.tensor_mul(dt_sb[:], sg[:, 0, :], dt_sb[:])
                    dtb = wpool.tile([128, BL], bf16, tag="dtb")
                    nc.vector.tensor_copy(dtb[:], dt_sb[:])
                    for hm in range(KH):
                        nc.tensor.matmul(
                            pz[:, NM + hm, :],
                            dc_sb[:, hm * 128 : (hm + 1) * 128],
                            dtb[:],
                            start=False,
                            stop=(hm == KH - 1),
                        )
                    # cell = gf*cell + gi*chat + dc
                    tmp = wpool.tile([128, KH, BL], f32, tag="tmp")
                    nc.vector.tensor_mul(tmp[:], sg[:, 1:9, :], th[:])
                    nc.vector.tensor_mul(cell_sb[:], sg[:, 9:17, :], cell_sb[:])
                    nc.vector.tensor_add(cell_sb[:], cell_sb[:], tmp[:])
                    nc.vector.tensor_add(cell_sb[:], cell_sb[:], pz[:, NM:NZ, :])
                    # hid = go * tanh(cell), written straight into the history
                    thc = wpool.tile([128, KH, BL], f32, tag="thc")
                    nc.scalar.activation(thc[:], cell_sb[:], AF.Tanh)
                    nc.vector.tensor_mul(
                        hist[:, :, t * BL : (t + 1) * BL], sg[:, 17:25, :], thc[:]
                    )

                # ---- GEMM2: logits = ow.T @ hist + ob, vocab streamed ----
                opool = cs.enter_context(tc.tile_pool(name="ow", bufs=2))
                ospool = cs.enter_context(tc.tile_pool(name="os", bufs=3))
                opsum = cs.enter_context(
                    tc.tile_pool(name="g2p", bufs=2, space="PSUM")
                )
                ccw = cols // 2
                for v0, nt in G2_CHUNKS:
                    ow_sb = opool.tile([128, KH, 8 * 128], bf16, tag="ow")
                    nc.sync.dma_start(
                        ow_sb[:, :, 0 : nt * 128],
                        owT.ap()[:, v0 * 128 : (v0 + nt) * 128].rearrange(
                            "(k p) m -> p k m", p=128
                        ),
                    )
                    for mi in range(nt):
                        m = v0 + mi
                        osb = ospool.tile([128, cols], f32, tag="osb")
                        for cc in range(2):
                            po = opsum.tile(
                                [128, ccw], f32, tag="po", name=f"po{m}_{cc}"
                            )
                            for k in range(KH):
                                nc.tensor.matmul(
                                    po[:],
                                    ow_sb[:, k, mi * 128 : (mi + 1) * 128],
                                    hist[:, k, cc * ccw : (cc + 1) * ccw],
                                    start=(k == 0),
                                    stop=(k == KH - 1),
                                )
                            nc.vector.tensor_scalar_add(
                                osb[:, cc * ccw : (cc + 1) * ccw],
                                po[:],
                                bo_sb[:, m : m + 1],
                            )
                        nc.sync.dma_start(outc.ap()[m], osb[:])

    nc.finalize()
    return nc


@functools.lru_cache(maxsize=2)
def _cached_module(t_steps=T, v_pad=V_PAD, nch=NCH, vs=V):
    return _build_module(t_steps, v_pad, nch, vs)


def _prep_inputs(
    input_seq, last_hidden, last_dt, w2h_w, w2h_b, h2h_w, h2h_b,
    w2h_r_w, w2h_r_b, h2h_r_w, h2h_r_b, dc_w, out_w, out_b,
):
    """Host-side sharding/layout. Returns per-core input dicts."""
    b, t_steps, v = input_seq.shape
    h = last_hidden.shape[1]
    d = last_dt.shape[1]
    cols = t_steps * BL
    v_pad = ((v + 127) // 128) * 128

    # weights (shared by all cores)
    w1cat = np.concatenate([w2h_r_w, w2h_w], axis=0)          # (4224, v)
    w1T = np.zeros((v_pad, NM * 128), np.float32)
    w1T[:v] = w1cat.T
    w1ch = np.ascontiguousarray(
        w1T.reshape(KV, 128, NM, 128).transpose(2, 1, 0, 3)
    ).astype(BF16)
    wcatT = np.ascontiguousarray(
        np.concatenate([(ALPHA * h2h_r_w).T, h2h_w.T], axis=1)
    ).astype(BF16)                                            # (h, 4224)
    dcT = np.ascontiguousarray(dc_w.T).astype(BF16)           # (d, h)
    owT = np.zeros((h, v_pad), np.float32)
    owT[:, :v] = out_w.T
    owT = owT.astype(BF16)
    biasG = np.zeros((128, NM), np.float32)
    biasG[:, 0] = w2h_r_b + ALPHA * h2h_r_b
    biasG[:, 1:] = (w2h_b + h2h_b).reshape(32, 128).T
    ob = np.zeros(v_pad, np.float32)
    ob[:v] = out_b
    biasO = np.ascontiguousarray(ob.reshape(NVT, 128).T)
    ident = np.eye(128, dtype=BF16)

    in_maps = []
    for c in range(NCORE):
        bs = slice(c * BL, (c + 1) * BL)
        xsT = np.zeros((v_pad, cols), np.float32)
        xr = xsT[:v].reshape(v, t_steps, BL)
        xr[:, 1:, :] = input_seq[bs].transpose(2, 1, 0)[:, : t_steps - 1, :]
        xr[0, 0, :] = 1.0  # SOS one-hot
        in_maps.append(
            {
                "xsT": xsT.astype(BF16),
                "w1ch": w1ch,
                "wcatT": wcatT,
                "dcT": dcT,
                "owT": owT,
                "biasG": biasG,
                "biasO": biasO,
                "identI": ident,
                "hidT0": np.ascontiguousarray(last_hidden[bs].T).astype(BF16),
                "cellT0": np.ascontiguousarray(last_hidden[bs].T).astype(
                    np.float32
                ),
                "dtT0": np.ascontiguousarray(last_dt[bs].T).astype(np.float32),
            }
        )
    return in_maps, cols, v_pad, v


def _assemble(results, t_steps=T, v=V):
    """Stack per-core outc tensors back into the full (B, T, V) output."""
    out = np.empty((B, t_steps, v), np.float32)
    for c in range(NCORE):
        o = np.asarray(results[c]["outc"])  # (NVT, 128, cols)
        out[c * BL : (c + 1) * BL] = (
            o.reshape(NVT, 128, t_steps, BL)
            .transpose(3, 2, 0, 1)
            .reshape(BL, t_steps, NVT * 128)[:, :, :v]
        )
    return out


def kernel(**inputs):
    from concourse.bass_utils import run_bass_kernel_spmd

    input_seq = np.asarray(inputs["input_seq"], np.float32)
    b, t_steps, v = input_seq.shape
    args = {
        k: np.asarray(inputs[k], np.float32)
        for k in (
            "last_hidden", "last_dt", "w2h_w", "w2h_b", "h2h_w", "h2h_b",
            "w2h_r_w", "w2h_r_b", "h2h_r_w", "h2h_r_b", "dc_w", "out_w", "out_b",
        )
    }
    in_maps, _, v_pad, _ = _prep_inputs(input_seq, **args)
    nc = _cached_module(t_steps, v_pad, t_steps * BL, v)
    res = run_bass_kernel_spmd(nc, in_maps, core_ids=list(range(NCORE)))
    return np.ascontiguousarray(_assemble(res.results, t_steps, v))


# revision 28
# speedup vs baseline: 2.7814x; 1.1780x over previous
"""Trainium2 Bass kernel for nn_Decoder (LSTM-style decoder with r/dt side path).

Reference math (per step t, teacher forcing):
    xs_t    = SOS one-hot (t=0) or input_seq[:, t-1]
    z       = xs_t @ w2h_w.T + w2h_b + hid @ h2h_w.T + h2h_b          (B, 4H)
    gi,gf,go = sigmoid(z[:, 0:H]), sigmoid(z[:, H:2H]), sigmoid(z[:, 2H:3H])
    chat    = tanh(z[:, 3H:4H])
    gr      = sigmoid(xs_t @ w2h_r_w.T + w2h_r_b + a*(hid @ h2h_r_w.T + h2h_r_b))
    dt      = gr * dt
    cell    = gf*cell + gi*chat + dt @ dc_w.T
    hid     = go * tanh(cell)
    logits  = hid @ out_w.T + out_b

Distribution: data-parallel over batch (the sharding_hint's primary option).
Each of the 8 cores runs 8 of the 64 sequences end-to-end with replicated
weights — no collectives and no cross-core synchronization anywhere.

Per-core schedule (features on partitions, (t, b_local) on free dims):
  GEMM1  pre = w1.T @ xs   (4224, 600): column half A runs up front; half B
         is sliced into small PE bursts interleaved between scan steps so
         the Tensor engine stays busy while the scan's activation / vector
         chain runs. w1 (66MB) streams through SBUF one row-tile at a time,
         once per column half.
  scan   75 steps. All 41 per-step PSUM slices (33 z-tiles + 8 dc-tiles)
         live in ONE 2KB PSUM bank: a single identity-matmul injection
         (start=True) pending-zeroes the bank and seeds z with pre[t];
         every following matmul accumulates with start=False into its own
         disjoint slice. Gate slices are ordered so r|gi|gf finish first
         and go last, shortening the serial chain. The hidden history stays
         resident in SBUF — no DRAM round-trip in the scan.
  GEMM2  logits = ow.T @ hist: the first column half interleaves with the
         remaining scan steps, the rest runs as a short tail.
"""

import functools

import numpy as np
import ml_dtypes

B = 64
T = 75
V = 8000
H = 1024
D = 128
ALPHA = 0.5
NCORE = 8
BL = B // NCORE          # 8: per-core batch
COLS = T * BL            # 600: per-core (t, b) columns
V_PAD = 8064             # 63 * 128
KV = V_PAD // 128        # 63 K-tiles for GEMM1
KH = H // 128            # 8 K-tiles for the scan / GEMM2
NM = 33                  # GEMM1 / z output row tiles: r(1) + gates(32)
NZ = NM + KH             # 41: z tiles + dc tiles share one PSUM bank
NVT = V_PAD // 128       # 63 vocab tiles
NCH = COLS               # kept for test.py compatibility
HW1 = 352                # columns computed before the scan starts (mult of BL)
HW2 = COLS - HW1

BF16 = ml_dtypes.bfloat16
# GEMM2 vocab chunks (tile_start, n_tiles)
G2_CHUNKS = [(i, min(8, NVT - i)) for i in range(0, NVT, 8)]
PE_NS = 0.4167           # full-speed PE ns per output column


class _Filler:
    """Queue of (cost_ns, emit_fn, min_step) Tensor-engine work, drained in
    budgeted slices between scan-step fragments so the PE never idles while
    the scan's activation/vector chain runs. Entries gated by min_step are
    skipped until the scan has produced the data they read."""

    def __init__(self):
        self.q = []
        self.head = 0

    def add(self, cost, fn, min_step=0):
        self.q.append((cost, fn, min_step))

    def total(self):
        return sum(c for c, _, _ in self.q)

    def emit(self, budget, step=1 << 30):
        while self.head < len(self.q) and budget > 0:
            cost, fn, min_step = self.q[self.head]
            if step < min_step:
                break
            self.head += 1
            fn()
            budget -= cost
        return budget

    def drain(self):
        self.emit(float("inf"))


def _build_module(t_steps=T, v_pad=V_PAD, nch=NCH, vs=V):
    import concourse.mybir as mybir
    import concourse.tile as tile
    from concourse import bacc

    dt_ = mybir.dt
    f32, bf16 = dt_.float32, dt_.bfloat16
    AF = mybir.ActivationFunctionType

    cols = t_steps * BL
    nt1 = HW1 // BL

    nc = bacc.Bacc("TRN2", target_bir_lowering=False, num_devices=NCORE)

    # ---------------- I/O ----------------
    xsT = nc.dram_tensor("xsT", [v_pad, cols], bf16, kind="ExternalInput")
    w1ch = nc.dram_tensor("w1ch", [NM, 128, KV, 128], bf16, kind="ExternalInput")
    wcatT = nc.dram_tensor("wcatT", [H, NM * 128], bf16, kind="ExternalInput")
    dcT = nc.dram_tensor("dcT", [D, H], f32, kind="ExternalInput")
    owT = nc.dram_tensor("owT", [H, v_pad], bf16, kind="ExternalInput")
    biasG = nc.dram_tensor("biasG", [128, NM], f32, kind="ExternalInput")
    biasO = nc.dram_tensor("biasO", [128, NVT], f32, kind="ExternalInput")
    identI = nc.dram_tensor("identI", [128, 128], bf16, kind="ExternalInput")
    hidT0 = nc.dram_tensor("hidT0", [H, BL], bf16, kind="ExternalInput")
    cellT0 = nc.dram_tensor("cellT0", [H, BL], f32, kind="ExternalInput")
    dtT0 = nc.dram_tensor("dtT0", [D, BL], f32, kind="ExternalInput")
    outc = nc.dram_tensor("outc", [NVT, 128, cols], f32, kind="ExternalOutput")

    with tile.TileContext(nc) as tc:
        import contextlib

        with contextlib.ExitStack() as ctx:
            cpool = ctx.enter_context(tc.tile_pool(name="const", bufs=1))
            spool = ctx.enter_context(tc.tile_pool(name="state", bufs=1))

            # resident constants / accumulators (tiles allocated up front;
            # their DMAs are deferred so GEMM1's xs/w1 loads go first and the
            # first matmul isn't stuck behind ~50us of constant transfers)
            pre = cpool.tile([128, t_steps, NM, BL], bf16)       # 38.7KB/part
            dc_sb = cpool.tile([128, H], f32)
            bg_sb = cpool.tile([128, NM], f32)
            bo_sb = cpool.tile([128, NVT], f32)
            id_sb = cpool.tile([128, 128], bf16)
            hid0_sb = spool.tile([128, KH, BL], bf16)
            cell_sb = spool.tile([128, KH, BL], f32)
            dt_sb = spool.tile([128, BL], f32)
            wcat_sb = cpool.tile([128, KH, NM * 128], bf16)      # 66KB/part
            hist = cpool.tile([128, KH, cols], bf16)             # 9.4KB/part

            def dma_const():
                yield lambda: nc.sync.dma_start(
                    hid0_sb[:], hidT0.ap().rearrange("(k p) n -> p k n", p=128)
                )
                yield lambda: nc.sync.dma_start(
                    cell_sb[:], cellT0.ap().rearrange("(k p) n -> p k n", p=128)
                )
                yield lambda: nc.sync.dma_start(dt_sb[:], dtT0.ap())
                yield lambda: nc.sync.dma_start(id_sb[:], identI.ap())
                # wcat (8.4MB) in per-k slices that slot into w1 DMA gaps
                for kk in range(KH):
                    yield lambda kk=kk: nc.sync.dma_start(
                        wcat_sb[:, kk, :],
                        wcatT.ap()[kk * 128 : (kk + 1) * 128, :],
                    )
                yield lambda: nc.sync.dma_start(dc_sb[:], dcT.ap())
                yield lambda: nc.sync.dma_start(bo_sb[:], biasO.ap())

            const_dmas = dma_const()
            wpool = ctx.enter_context(tc.tile_pool(name="work", bufs=2))
            zpool = ctx.enter_context(
                tc.tile_pool(name="zp", bufs=2, space="PSUM")
            )

            evict_flip = {"v": 0}

            def evict(dst, src, bias):
                # alternate the psum->sbuf bias-add between DVE and Act so
                # neither engine's scan-chain work queues behind evictions
                evict_flip["v"] ^= 1
                if evict_flip["v"]:
                    nc.vector.tensor_scalar_add(dst, src, bias)
                else:
                    nc.scalar.activation(dst, src, AF.Identity, bias=bias)

            def scan_a(t):
                # one bank: [r|gi|gf|go|chat](33) + dc(8), all f32 x BL
                pz = zpool.tile([128, NZ, BL], f32, tag="z", name=f"z{t}")
                # identity injection seeds z with pre[t] and pending-zeroes
                # the whole bank (incl. the dc slices)
                nc.tensor.matmul(
                    pz[:, 0:NM, :], id_sb[:], pre[:, t, :, :],
                    start=True, stop=False,
                )

                def rhs(k):
                    return (
                        hid0_sb[:, k, :]
                        if t == 0
                        else hist[:, k, (t - 1) * BL : t * BL]
                    )

                # m-outer so early slices complete first: r|gi|gf feed the dt
                # and cell chains, chat feeds gi*chat, go is needed last.
                for m in list(range(17)) + list(range(25, NM)) + list(range(17, 25)):
                    for k in range(KH):
                        nc.tensor.matmul(
                            pz[:, m, :],
                            wcat_sb[:, k, m * 128 : (m + 1) * 128],
                            rhs(k),
                            start=False,
                            stop=False,
                        )
                sg = wpool.tile([128, 25, BL], f32, tag="sg")
                th = wpool.tile([128, KH, BL], f32, tag="th")
                nc.scalar.activation(sg[:, 0:17, :], pz[:, 0:17, :], AF.Sigmoid)
                nc.scalar.activation(th[:], pz[:, 25:NM, :], AF.Tanh)
                nc.scalar.activation(sg[:, 17:25, :], pz[:, 17:25, :], AF.Sigmoid)
                nc.vector.tensor_mul(dt_sb[:], sg[:, 0, :], dt_sb[:])
                return pz, sg, th

            def scan_b(t, pz, sg, th):
                # dc = dc_w @ dt accumulated into the bank (f32 operands)
                for hm in range(KH):
                    nc.tensor.matmul(
                        pz[:, NM + hm, :],
                        dc_sb[:, hm * 128 : (hm + 1) * 128],
                        dt_sb[:],
                        start=False,
                        stop=(hm == KH - 1),
                    )
                # cell = gf*cell + gi*chat + dc
                tmp = wpool.tile([128, KH, BL], f32, tag="tmp")
                nc.vector.tensor_mul(cell_sb[:], sg[:, 9:17, :], cell_sb[:])
                nc.vector.tensor_mul(tmp[:], sg[:, 1:9, :], th[:])
                nc.vector.tensor_add(cell_sb[:], cell_sb[:], tmp[:])
                nc.vector.tensor_add(cell_sb[:], cell_sb[:], pz[:, NM:NZ, :])
                # hid = go * tanh(cell), written straight into the history
                thc = wpool.tile([128, KH, BL], f32, tag="thc")
                nc.scalar.activation(thc[:], cell_sb[:], AF.Tanh)
                nc.vector.tensor_mul(
                    hist[:, :, t * BL : (t + 1) * BL], sg[:, 17:25, :], thc[:]
                )

            # ---- GEMM1 phase A + interleaved phase B ----
            with contextlib.ExitStack() as c1:
                xpool = c1.enter_context(tc.tile_pool(name="xs", bufs=1))
                w1pool = c1.enter_context(tc.tile_pool(name="w1", bufs=2))
                gpsum = c1.enter_context(
                    tc.tile_pool(name="g1p", bufs=2, space="PSUM")
                )

                w1_tiles = {}

                def w1_dma(u):
                    if u in w1_tiles or u >= 2 * NM:
                        return
                    w1sb = w1pool.tile([128, KV, 128], bf16, tag="w1")
                    nc.sync.dma_start(w1sb[:], w1ch.ap()[u % NM])
                    w1_tiles[u] = w1sb

                # xs half A in two k-slices so unit 0 starts ~7us earlier
                xs_a = xpool.tile([128, KV, HW1], bf16, tag="xs")
                nc.sync.dma_start(
                    xs_a[:, 0:32, :],
                    xsT.ap()[: 32 * 128, 0:HW1].rearrange(
                        "(k p) n -> p k n", p=128
                    ),
                )
                w1_dma(0)
                nc.sync.dma_start(
                    xs_a[:, 32:KV, :],
                    xsT.ap()[32 * 128 :, 0:HW1].rearrange(
                        "(k p) n -> p k n", p=128
                    ),
                )
                nc.sync.dma_start(bg_sb[:], biasG.ap())
                for u in range(NM):
                    w1_dma(u)
                    w1_dma(u + 1)
                    pg = gpsum.tile([128, HW1], f32, tag="pg", name=f"pga{u}")
                    for k in range(KV):
                        nc.tensor.matmul(
                            pg[:],
                            w1_tiles[u][:, k, :],
                            xs_a[:, k, :],
                            start=(k == 0),
                            stop=(k == KV - 1),
                        )
                    evict(pre[:, 0:nt1, u, :], pg[:], bg_sb[:, u : u + 1])
                    w1_tiles.pop(u, None)
                    # slot one deferred constant DMA behind each unit so they
                    # fill w1-stream gaps without delaying the w1 prefetches
                    if u >= 1:
                        fn = next(const_dmas, None)
                        if fn is not None:
                            fn()

                # phase B input (reuses the xs buffer; WAR-serialized by
                # Tile). Loaded in k-slices so the first B units start as
                # soon as their k-range has landed.
                xs_b = xpool.tile([128, KV, HW1], bf16, tag="xs")
                for k0, nk in ((0, 21), (21, 21), (42, 21)):
                    nc.sync.dma_start(
                        xs_b[:, k0 : k0 + nk, 0:HW2],
                        xsT.ap()[k0 * 128 : (k0 + nk) * 128, HW1:cols].rearrange(
                            "(k p) n -> p k n", p=128
                        ),
                    )

                # queue phase-B units as ~2.6us k-slices
                g1fill = _Filler()
                KSPLIT = [(0, 21), (21, 21), (42, 21)]

                def g1b_slice(m, k0, nk):
                    def emit():
                        if k0 == 0:
                            w1_dma(NM + m)
                            w1_dma(NM + m + 1)
                        pg = g1fill.pg if k0 else gpsum.tile(
                            [128, HW1], f32, tag="pg", name=f"pgb{m}"
                        )
                        g1fill.pg = pg
                        for k in range(k0, k0 + nk):
                            nc.tensor.matmul(
                                pg[:, 0:HW2],
                                w1_tiles[NM + m][:, k, :],
                                xs_b[:, k, 0:HW2],
                                start=(k == 0),
                                stop=(k == KV - 1),
                            )
                        if k0 + nk == KV:
                            evict(
                                pre[:, nt1 : cols // BL, m, :],
                                pg[:, 0:HW2],
                                bg_sb[:, m : m + 1],
                            )
                            w1_tiles.pop(NM + m, None)

                    return emit

                for m in range(NM):
                    for k0, nk in KSPLIT:
                        g1fill.add(nk * HW2 * PE_NS, g1b_slice(m, k0, nk))

                quota = g1fill.total() / max(1, nt1 - 2)
                deficit = 0.0
                for t in range(min(nt1, t_steps)):
                    deficit += quota
                    pz, sg, th = scan_a(t)
                    deficit = g1fill.emit(deficit * 0.55) + deficit * 0.45
                    scan_b(t, pz, sg, th)
                    deficit = g1fill.emit(deficit)
                g1fill.drain()

            # ---- GEMM2 pass A + scan steps nt1..T-1, then the tail ----
            with contextlib.ExitStack() as c2:
                opool = c2.enter_context(tc.tile_pool(name="ow", bufs=2))
                ospool = c2.enter_context(tc.tile_pool(name="os", bufs=2))
                opsum = c2.enter_context(
                    tc.tile_pool(name="g2p", bufs=2, space="PSUM")
                )

                ow_tiles = {}

                def ow_dma(ci):
                    if ci in ow_tiles or not (0 <= ci < len(G2_CHUNKS)):
                        return
                    v0, nt = G2_CHUNKS[ci]
                    ow_sb = opool.tile([128, KH, 8 * 128], bf16, tag="ow")
                    nc.sync.dma_start(
                        ow_sb[:, :, 0 : nt * 128],
                        owT.ap()[:, v0 * 128 : (v0 + nt) * 128].rearrange(
                            "(k p) m -> p k m", p=128
                        ),
                    )
                    ow_tiles[ci] = ow_sb

                osb_cur = {}

                def g2_unit(ci, mi, h0, hw, last, prefetch):
                    def emit():
                        if mi == 0:
                            ow_dma(ci)
                            osb_cur["t"] = ospool.tile(
                                [128, 8, HW1], f32, tag="osb",
                                name=f"osb{h0}_{ci}",
                            )
                        if mi == 2:
                            ow_dma(prefetch)
                        v0, nt = G2_CHUNKS[ci]
                        m = v0 + mi
                        po = opsum.tile(
                            [128, HW1], f32, tag="po", name=f"po{h0}_{m}"
                        )
                        for k in range(KH):
                            nc.tensor.matmul(
                                po[:, 0:hw],
                                ow_tiles[ci][:, k, mi * 128 : (mi + 1) * 128],
                                hist[:, k, h0 : h0 + hw],
                                start=(k == 0),
                                stop=(k == KH - 1),
                            )
                        osb = osb_cur["t"]
                        evict(
                            osb[:, mi, 0:hw], po[:, 0:hw], bo_sb[:, m : m + 1]
                        )
                        if mi == nt - 1:
                            # one batched DMA for the whole vocab chunk
                            nc.sync.dma_start(
                                outc.ap()[v0 : v0 + nt][
                                    :, :, h0 : h0 + hw
                                ].rearrange("m p n -> p m n"),
                                osb[:, 0:nt, 0:hw],
                            )
                        if last:
                            ow_tiles.pop(ci, None)

                    return emit

                # Two column passes over the vocab: [0:HW1] interleaves with
                # the remaining scan steps, [HW1:] is the tail. Chunk order
                # alternates per pass so the chunk left resident at the pass
                # boundary is reused without a re-DMA.
                g2fill = _Filler()
                nch = len(G2_CHUNKS)

                def add_pass(order, h0, hw, min_step, filler=None, keep_last=False):
                    for j, ci in enumerate(order):
                        v0, nt = G2_CHUNKS[ci]
                        is_last = j + 1 == len(order)
                        nxt = -1 if is_last else order[j + 1]
                        for mi in range(nt):
                            u = g2_unit(
                                ci, mi, h0, hw,
                                last=(mi == nt - 1 and not (is_last and keep_last)),
                                prefetch=nxt,
                            )
                            if filler is None:
                                u()
                            else:
                                filler.add(KH * hw * PE_NS, u, min_step)

                fwd = list(range(nch))
                rev = list(reversed(fwd))
                add_pass(fwd, 0, HW1, nt1, g2fill, keep_last=True)

                quota = g2fill.total() / max(1, t_steps - nt1 - 2)
                deficit = 0.0
                for t in range(nt1, t_steps):
                    deficit += quota
                    pz, sg, th = scan_a(t)
                    deficit = g2fill.emit(deficit * 0.55, t) + deficit * 0.45
                    scan_b(t, pz, sg, th)
                    deficit = g2fill.emit(deficit, t)
                g2fill.drain()

                # tail: remaining columns, reusing the resident last chunk
                add_pass(rev, HW1, HW2, 0, None)

    nc.finalize()
    return nc


@functools.lru_cache(maxsize=2)
def _cached_module(t_steps=T, v_pad=V_PAD, nch=NCH, vs=V):
    return _build_module(t_steps, v_pad, nch, vs)


def _prep_inputs(
    input_seq, last_hidden, last_dt, w2h_w, w2h_b, h2h_w, h2h_b,
    w2h_r_w, w2h_r_b, h2h_r_w, h2h_r_b, dc_w, out_w, out_b,
):
    """Host-side sharding/layout. Returns per-core input dicts."""
    b, t_steps, v = input_seq.shape
    h = last_hidden.shape[1]
    d = last_dt.shape[1]
    cols = t_steps * BL
    v_pad = ((v + 127) // 128) * 128

    # weights (shared by all cores)
    w1cat = np.concatenate([w2h_r_w, w2h_w], axis=0)          # (4224, v)
    w1T = np.zeros((v_pad, NM * 128), np.float32)
    w1T[:v] = w1cat.T
    w1ch = np.ascontiguousarray(
        w1T.reshape(KV, 128, NM, 128).transpose(2, 1, 0, 3)
    ).astype(BF16)
    wcatT = np.ascontiguousarray(
        np.concatenate([(ALPHA * h2h_r_w).T, h2h_w.T], axis=1)
    ).astype(BF16)                                            # (h, 4224)
    dcT = np.ascontiguousarray(dc_w.T).astype(np.float32)     # (d, h)
    owT = np.zeros((h, v_pad), np.float32)
    owT[:, :v] = out_w.T
    owT = owT.astype(BF16)
    biasG = np.zeros((128, NM), np.float32)
    biasG[:, 0] = w2h_r_b + ALPHA * h2h_r_b
    biasG[:, 1:] = (w2h_b + h2h_b).reshape(32, 128).T
    ob = np.zeros(v_pad, np.float32)
    ob[:v] = out_b
    biasO = np.ascontiguousarray(ob.reshape(NVT, 128).T)
    ident = np.eye(128, dtype=BF16)

    in_maps = []
    for c in range(NCORE):
        bs = slice(c * BL, (c + 1) * BL)
        xsT = np.zeros((v_pad, cols), np.float32)
        xr = xsT[:v].reshape(v, t_steps, BL)
        xr[:, 1:, :] = input_seq[bs].transpose(2, 1, 0)[:, : t_steps - 1, :]
        xr[0, 0, :] = 1.0  # SOS one-hot
        in_maps.append(
            {
                "xsT": xsT.astype(BF16),
                "w1ch": w1ch,
                "wcatT": wcatT,
                "dcT": dcT,
                "owT": owT,
                "biasG": biasG,
                "biasO": biasO,
                "identI": ident,
                "hidT0": np.ascontiguousarray(last_hidden[bs].T).astype(BF16),
                "cellT0": np.ascontiguousarray(last_hidden[bs].T).astype(
                    np.float32
                ),
                "dtT0": np.ascontiguousarray(last_dt[bs].T).astype(np.float32),
            }
        )
    return in_maps, cols, v_pad, v


def _assemble(results, t_steps=T, v=V):
    """Stack per-core outc tensors back into the full (B, T, V) output."""
    out = np.empty((B, t_steps, v), np.float32)
    for c in range(NCORE):
        o = np.asarray(results[c]["outc"])  # (NVT, 128, cols)
        out[c * BL : (c + 1) * BL] = (
            o.reshape(NVT, 128, t_steps, BL)
            .transpose(3, 2, 0, 1)
            .reshape(BL, t_steps, NVT * 128)[:, :, :v]
        )
    return out


def kernel(**inputs):
    from concourse.bass_utils import run_bass_kernel_spmd

    input_seq = np.asarray(inputs["input_seq"], np.float32)
    b, t_steps, v = input_seq.shape
    args = {
        k: np.asarray(inputs[k], np.float32)
        for k in (
            "last_hidden", "last_dt", "w2h_w", "w2h_b", "h2h_w", "h2h_b",
            "w2h_r_w", "w2h_r_b", "h2h_r_w", "h2h_r_b", "dc_w", "out_w", "out_b",
        )
    }
    in_maps, _, v_pad, _ = _prep_inputs(input_seq, **args)
    nc = _cached_module(t_steps, v_pad, t_steps * BL, v)
    res = run_bass_kernel_spmd(nc, in_maps, core_ids=list(range(NCORE)))
    return np.ascontiguousarray(_assemble(res.results, t_steps, v))


# revision 29
# speedup vs baseline: 2.9482x; 1.0600x over previous
"""Trainium2 Bass kernel for nn_Decoder (LSTM-style decoder with r/dt side path).

Reference math (per step t, teacher forcing):
    xs_t    = SOS one-hot (t=0) or input_seq[:, t-1]
    z       = xs_t @ w2h_w.T + w2h_b + hid @ h2h_w.T + h2h_b          (B, 4H)
    gi,gf,go = sigmoid(z[:, 0:H]), sigmoid(z[:, H:2H]), sigmoid(z[:, 2H:3H])
    chat    = tanh(z[:, 3H:4H])
    gr      = sigmoid(xs_t @ w2h_r_w.T + w2h_r_b + a*(hid @ h2h_r_w.T + h2h_r_b))
    dt      = gr * dt
    cell    = gf*cell + gi*chat + dt @ dc_w.T
    hid     = go * tanh(cell)
    logits  = hid @ out_w.T + out_b

Distribution: data-parallel over batch (the sharding_hint's primary option).
Each of the 8 cores runs 8 of the 64 sequences end-to-end with replicated
weights — no collectives and no cross-core synchronization anywhere.

Per-core schedule (features on partitions, (t, b_local) on free dims):
  GEMM1  pre = w1.T @ xs   (4224, 600): column half A runs up front; half B
         is sliced into small PE bursts interleaved between scan steps so
         the Tensor engine stays busy while the scan's activation / vector
         chain runs. w1 (66MB) streams through SBUF one row-tile at a time,
         once per column half.
  scan   75 steps. All 41 per-step PSUM slices (33 z-tiles + 8 dc-tiles)
         live in ONE 2KB PSUM bank: a single identity-matmul injection
         (start=True) pending-zeroes the bank and seeds z with pre[t];
         every following matmul accumulates with start=False into its own
         disjoint slice. Gate slices are ordered so r|gi|gf finish first
         and go last, shortening the serial chain. The hidden history stays
         resident in SBUF — no DRAM round-trip in the scan.
  GEMM2  logits = ow.T @ hist: the first column half interleaves with the
         remaining scan steps, the rest runs as a short tail.
"""

import functools

import numpy as np
import ml_dtypes

B = 64
T = 75
V = 8000
H = 1024
D = 128
ALPHA = 0.5
NCORE = 8
BL = B // NCORE          # 8: per-core batch
COLS = T * BL            # 600: per-core (t, b) columns
V_PAD = 8064             # 63 * 128
KV = V_PAD // 128        # 63 K-tiles for GEMM1
KH = H // 128            # 8 K-tiles for the scan / GEMM2
NM = 33                  # GEMM1 / z output row tiles: r(1) + gates(32)
NZ = NM + KH             # 41: z tiles + dc tiles share one PSUM bank
NVT = V_PAD // 128       # 63 vocab tiles
NCH = COLS               # kept for test.py compatibility
HW1 = 352                # columns computed before the scan starts (mult of BL)
HW2 = COLS - HW1

BF16 = ml_dtypes.bfloat16
# GEMM2 vocab chunks (tile_start, n_tiles)
G2_CHUNKS = [(i, min(8, NVT - i)) for i in range(0, NVT, 8)]
PE_NS = 0.4167           # full-speed PE ns per output column


class _Filler:
    """Queue of (cost_ns, emit_fn, min_step) Tensor-engine work, drained in
    budgeted slices between scan-step fragments so the PE never idles while
    the scan's activation/vector chain runs. Entries gated by min_step are
    skipped until the scan has produced the data they read."""

    def __init__(self):
        self.q = []
        self.head = 0

    def add(self, cost, fn, min_step=0):
        self.q.append((cost, fn, min_step))

    def total(self):
        return sum(c for c, _, _ in self.q)

    def emit(self, budget, step=1 << 30):
        while self.head < len(self.q) and budget > 0:
            cost, fn, min_step = self.q[self.head]
            if step < min_step:
                break
            self.head += 1
            fn()
            budget -= cost
        return budget

    def drain(self):
        self.emit(float("inf"))


def _build_module(t_steps=T, v_pad=V_PAD, nch=NCH, vs=V):
    import concourse.mybir as mybir
    import concourse.tile as tile
    from concourse import bacc

    dt_ = mybir.dt
    f32, bf16 = dt_.float32, dt_.bfloat16
    AF = mybir.ActivationFunctionType

    cols = t_steps * BL
    nt1 = HW1 // BL

    nc = bacc.Bacc("TRN2", target_bir_lowering=False, num_devices=NCORE)

    # ---------------- I/O ----------------
    xsT = nc.dram_tensor("xsT", [v_pad, cols], bf16, kind="ExternalInput")
    w1ch = nc.dram_tensor("w1ch", [NM, 128, KV, 128], bf16, kind="ExternalInput")
    wcatT = nc.dram_tensor("wcatT", [H, NM * 128], bf16, kind="ExternalInput")
    dcT = nc.dram_tensor("dcT", [D, H], f32, kind="ExternalInput")
    owT = nc.dram_tensor("owT", [H, v_pad], bf16, kind="ExternalInput")
    biasG = nc.dram_tensor("biasG", [128, NM], f32, kind="ExternalInput")
    biasO = nc.dram_tensor("biasO", [128, NVT], f32, kind="ExternalInput")
    identI = nc.dram_tensor("identI", [128, 128], bf16, kind="ExternalInput")
    hidT0 = nc.dram_tensor("hidT0", [H, BL], bf16, kind="ExternalInput")
    cellT0 = nc.dram_tensor("cellT0", [H, BL], f32, kind="ExternalInput")
    dtT0 = nc.dram_tensor("dtT0", [D, BL], f32, kind="ExternalInput")
    outc = nc.dram_tensor("outc", [NVT, 128, cols], f32, kind="ExternalOutput")

    with tile.TileContext(nc) as tc:
        import contextlib

        with contextlib.ExitStack() as ctx:
            cpool = ctx.enter_context(tc.tile_pool(name="const", bufs=1))
            spool = ctx.enter_context(tc.tile_pool(name="state", bufs=1))

            # resident constants / accumulators (tiles allocated up front;
            # their DMAs are deferred so GEMM1's xs/w1 loads go first and the
            # first matmul isn't stuck behind ~50us of constant transfers)
            pre = cpool.tile([128, t_steps, NM, BL], bf16)       # 38.7KB/part
            dc_sb = cpool.tile([128, H], f32)
            bg_sb = cpool.tile([128, NM], f32)
            bo_sb = cpool.tile([128, NVT], f32)
            id_sb = cpool.tile([128, 128], bf16)
            hid0_sb = spool.tile([128, KH, BL], bf16)
            cell_sb = spool.tile([128, KH, BL], f32)
            dt_sb = spool.tile([128, BL], f32)
            wcat_sb = cpool.tile([128, KH, NM * 128], bf16)      # 66KB/part
            hist = cpool.tile([128, KH, cols], bf16)             # 9.4KB/part

            def dma_const():
                yield lambda: nc.sync.dma_start(
                    hid0_sb[:], hidT0.ap().rearrange("(k p) n -> p k n", p=128)
                )
                yield lambda: nc.sync.dma_start(
                    cell_sb[:], cellT0.ap().rearrange("(k p) n -> p k n", p=128)
                )
                yield lambda: nc.sync.dma_start(dt_sb[:], dtT0.ap())
                yield lambda: nc.sync.dma_start(id_sb[:], identI.ap())
                # wcat (8.4MB) in per-k slices that slot into w1 DMA gaps
                for kk in range(KH):
                    yield lambda kk=kk: nc.sync.dma_start(
                        wcat_sb[:, kk, :],
                        wcatT.ap()[kk * 128 : (kk + 1) * 128, :],
                    )
                yield lambda: nc.sync.dma_start(dc_sb[:], dcT.ap())
                yield lambda: nc.sync.dma_start(bo_sb[:], biasO.ap())

            const_dmas = dma_const()
            wpool = ctx.enter_context(tc.tile_pool(name="work", bufs=2))
            zpool = ctx.enter_context(
                tc.tile_pool(name="zp", bufs=2, space="PSUM")
            )

            evict_flip = {"v": 0}

            def evict(dst, src, bias):
                # alternate the psum->sbuf bias-add between DVE and Act so
                # neither engine's scan-chain work queues behind evictions
                evict_flip["v"] ^= 1
                if evict_flip["v"]:
                    nc.vector.tensor_scalar_add(dst, src, bias)
                else:
                    nc.scalar.activation(dst, src, AF.Identity, bias=bias)

            def scan_a(t):
                # one bank: [r|gi|gf|go|chat](33) + dc(8), all f32 x BL
                pz = zpool.tile([128, NZ, BL], f32, tag="z", name=f"z{t}")
                # identity injection seeds z with pre[t] and pending-zeroes
                # the whole bank (incl. the dc slices)
                nc.tensor.matmul(
                    pz[:, 0:NM, :], id_sb[:], pre[:, t, :, :],
                    start=True, stop=False,
                )

                def rhs(k):
                    return (
                        hid0_sb[:, k, :]
                        if t == 0
                        else hist[:, k, (t - 1) * BL : t * BL]
                    )

                # m-outer so early slices complete first: r|gi|gf feed the dt
                # and cell chains, chat feeds gi*chat, go is needed last.
                for m in list(range(17)) + list(range(25, NM)) + list(range(17, 25)):
                    for k in range(KH):
                        nc.tensor.matmul(
                            pz[:, m, :],
                            wcat_sb[:, k, m * 128 : (m + 1) * 128],
                            rhs(k),
                            start=False,
                            stop=False,
                        )
                sg = wpool.tile([128, 25, BL], f32, tag="sg")
                th = wpool.tile([128, KH, BL], f32, tag="th")
                nc.scalar.activation(sg[:, 0:17, :], pz[:, 0:17, :], AF.Sigmoid)
                nc.scalar.activation(th[:], pz[:, 25:NM, :], AF.Tanh)
                nc.scalar.activation(sg[:, 17:25, :], pz[:, 17:25, :], AF.Sigmoid)
                nc.vector.tensor_mul(dt_sb[:], sg[:, 0, :], dt_sb[:])
                return pz, sg, th

            def scan_b(t, pz, sg, th):
                # dc = dc_w @ dt accumulated into the bank (f32 operands)
                for hm in range(KH):
                    nc.tensor.matmul(
                        pz[:, NM + hm, :],
                        dc_sb[:, hm * 128 : (hm + 1) * 128],
                        dt_sb[:],
                        start=False,
                        stop=(hm == KH - 1),
                    )
                # cell = gf*cell + gi*chat + dc
                tmp = wpool.tile([128, KH, BL], f32, tag="tmp")
                nc.vector.tensor_mul(cell_sb[:], sg[:, 9:17, :], cell_sb[:])
                nc.vector.tensor_mul(tmp[:], sg[:, 1:9, :], th[:])
                nc.vector.tensor_add(cell_sb[:], cell_sb[:], tmp[:])
                nc.vector.tensor_add(cell_sb[:], cell_sb[:], pz[:, NM:NZ, :])
                # hid = go * tanh(cell), written straight into the history
                thc = wpool.tile([128, KH, BL], f32, tag="thc")
                nc.scalar.activation(thc[:], cell_sb[:], AF.Tanh)
                nc.vector.tensor_mul(
                    hist[:, :, t * BL : (t + 1) * BL], sg[:, 17:25, :], thc[:]
                )

            # ---- GEMM1 phase A + interleaved phase B ----
            with contextlib.ExitStack() as c1:
                xpool = c1.enter_context(tc.tile_pool(name="xs", bufs=1))
                w1pool = c1.enter_context(tc.tile_pool(name="w1", bufs=2))
                gpsum = c1.enter_context(
                    tc.tile_pool(name="g1p", bufs=2, space="PSUM")
                )

                w1_tiles = {}

                def w1_dma(u):
                    if u in w1_tiles or u >= 2 * NM:
                        return
                    w1sb = w1pool.tile([128, KV, 128], bf16, tag="w1")
                    nc.sync.dma_start(w1sb[:], w1ch.ap()[u % NM])
                    w1_tiles[u] = w1sb

                # xs half A in k-slices so unit 0 starts as early as possible
                xs_a = xpool.tile([128, KV, HW1], bf16, tag="xs")
                nc.sync.dma_start(
                    xs_a[:, 0:12, :],
                    xsT.ap()[: 12 * 128, 0:HW1].rearrange(
                        "(k p) n -> p k n", p=128
                    ),
                )
                w1_dma(0)
                for k0, k1 in ((12, 28), (28, 45), (45, KV)):
                    nc.sync.dma_start(
                        xs_a[:, k0:k1, :],
                        xsT.ap()[k0 * 128 : k1 * 128, 0:HW1].rearrange(
                            "(k p) n -> p k n", p=128
                        ),
                    )
                nc.sync.dma_start(bg_sb[:], biasG.ap())
                for u in range(NM):
                    w1_dma(u)
                    w1_dma(u + 1)
                    pg = gpsum.tile([128, HW1], f32, tag="pg", name=f"pga{u}")
                    for k in range(KV):
                        nc.tensor.matmul(
                            pg[:],
                            w1_tiles[u][:, k, :],
                            xs_a[:, k, :],
                            start=(k == 0),
                            stop=(k == KV - 1),
                        )
                    evict(pre[:, 0:nt1, u, :], pg[:], bg_sb[:, u : u + 1])
                    w1_tiles.pop(u, None)
                    # slot one deferred constant DMA behind each unit so they
                    # fill w1-stream gaps without delaying the w1 prefetches
                    if u >= 1:
                        fn = next(const_dmas, None)
                        if fn is not None:
                            fn()

                # phase B input (reuses the xs buffer; WAR-serialized by
                # Tile). Loaded in k-slices so the first B units start as
                # soon as their k-range has landed.
                xs_b = xpool.tile([128, KV, HW1], bf16, tag="xs")
                for k0, nk in ((0, 21), (21, 21), (42, 21)):
                    nc.sync.dma_start(
                        xs_b[:, k0 : k0 + nk, 0:HW2],
                        xsT.ap()[k0 * 128 : (k0 + nk) * 128, HW1:cols].rearrange(
                            "(k p) n -> p k n", p=128
                        ),
                    )

                # queue phase-B units as ~2.6us k-slices
                g1fill = _Filler()
                KSPLIT = [(0, 21), (21, 21), (42, 21)]

                def g1b_slice(m, k0, nk):
                    def emit():
                        if k0 == 0:
                            w1_dma(NM + m)
                            w1_dma(NM + m + 1)
                        pg = g1fill.pg if k0 else gpsum.tile(
                            [128, HW1], f32, tag="pg", name=f"pgb{m}"
                        )
                        g1fill.pg = pg
                        for k in range(k0, k0 + nk):
                            nc.tensor.matmul(
                                pg[:, 0:HW2],
                                w1_tiles[NM + m][:, k, :],
                                xs_b[:, k, 0:HW2],
                                start=(k == 0),
                                stop=(k == KV - 1),
                            )
                        if k0 + nk == KV:
                            evict(
                                pre[:, nt1 : cols // BL, m, :],
                                pg[:, 0:HW2],
                                bg_sb[:, m : m + 1],
                            )
                            w1_tiles.pop(NM + m, None)

                    return emit

                for m in range(NM):
                    for k0, nk in KSPLIT:
                        g1fill.add(nk * HW2 * PE_NS, g1b_slice(m, k0, nk))

                quota = g1fill.total() / max(1, nt1 - 2)
                deficit = 0.0
                for t in range(min(nt1, t_steps)):
                    deficit += quota
                    pz, sg, th = scan_a(t)
                    deficit = g1fill.emit(deficit * 0.55) + deficit * 0.45
                    scan_b(t, pz, sg, th)
                    deficit = g1fill.emit(deficit)
                g1fill.drain()

            # ---- GEMM2 pass A + scan steps nt1..T-1, then the tail ----
            with contextlib.ExitStack() as c2:
                opool = c2.enter_context(tc.tile_pool(name="ow", bufs=2))
                ospool = c2.enter_context(tc.tile_pool(name="os", bufs=2))
                opsum = c2.enter_context(
                    tc.tile_pool(name="g2p", bufs=2, space="PSUM")
                )

                ow_tiles = {}

                def ow_dma(ci):
                    if ci in ow_tiles or not (0 <= ci < len(G2_CHUNKS)):
                        return
                    v0, nt = G2_CHUNKS[ci]
                    ow_sb = opool.tile([128, KH, 8 * 128], bf16, tag="ow")
                    nc.sync.dma_start(
                        ow_sb[:, :, 0 : nt * 128],
                        owT.ap()[:, v0 * 128 : (v0 + nt) * 128].rearrange(
                            "(k p) m -> p k m", p=128
                        ),
                    )
                    ow_tiles[ci] = ow_sb

                osb_cur = {}

                def g2_unit(ci, mi, h0, hw, last, prefetch):
                    def emit():
                        if mi == 0:
                            ow_dma(ci)
                            osb_cur["t"] = ospool.tile(
                                [128, 8, HW1], f32, tag="osb",
                                name=f"osb{h0}_{ci}",
                            )
                        if mi == 2:
                            ow_dma(prefetch)
                        v0, nt = G2_CHUNKS[ci]
                        m = v0 + mi
                        po = opsum.tile(
                            [128, HW1], f32, tag="po", name=f"po{h0}_{m}"
                        )
                        for k in range(KH):
                            nc.tensor.matmul(
                                po[:, 0:hw],
                                ow_tiles[ci][:, k, mi * 128 : (mi + 1) * 128],
                                hist[:, k, h0 : h0 + hw],
                                start=(k == 0),
                                stop=(k == KH - 1),
                            )
                        osb = osb_cur["t"]
                        evict(
                            osb[:, mi, 0:hw], po[:, 0:hw], bo_sb[:, m : m + 1]
                        )
                        if mi == nt - 1:
                            # one batched DMA for the whole vocab chunk
                            nc.sync.dma_start(
                                outc.ap()[v0 : v0 + nt][
                                    :, :, h0 : h0 + hw
                                ].rearrange("m p n -> p m n"),
                                osb[:, 0:nt, 0:hw],
                            )
                        if last:
                            ow_tiles.pop(ci, None)

                    return emit

                # Two column passes over the vocab: [0:HW1] interleaves with
                # the remaining scan steps, [HW1:] is the tail. Chunk order
                # alternates per pass so the chunk left resident at the pass
                # boundary is reused without a re-DMA.
                g2fill = _Filler()
                nch = len(G2_CHUNKS)

                def add_pass(order, h0, hw, min_step, filler=None, keep_last=False):
                    for j, ci in enumerate(order):
                        v0, nt = G2_CHUNKS[ci]
                        is_last = j + 1 == len(order)
                        nxt = -1 if is_last else order[j + 1]
                        for mi in range(nt):
                            u = g2_unit(
                                ci, mi, h0, hw,
                                last=(mi == nt - 1 and not (is_last and keep_last)),
                                prefetch=nxt,
                            )
                            if filler is None:
                                u()
                            else:
                                filler.add(KH * hw * PE_NS, u, min_step)

                fwd = list(range(nch))
                rev = list(reversed(fwd))
                add_pass(fwd, 0, HW1, nt1, g2fill, keep_last=True)

                quota = g2fill.total() / max(1, t_steps - nt1 - 2)
                deficit = 0.0
                for t in range(nt1, t_steps):
                    deficit += quota
                    pz, sg, th = scan_a(t)
                    deficit = g2fill.emit(deficit * 0.55, t) + deficit * 0.45
                    scan_b(t, pz, sg, th)
                    deficit = g2fill.emit(deficit, t)
                g2fill.drain()

                # tail: remaining columns, reusing the resident last chunk
                add_pass(rev, HW1, HW2, 0, None)

    nc.finalize()
    return nc


@functools.lru_cache(maxsize=2)
def _cached_module(t_steps=T, v_pad=V_PAD, nch=NCH, vs=V):
    return _build_module(t_steps, v_pad, nch, vs)


def _prep_inputs(
    input_seq, last_hidden, last_dt, w2h_w, w2h_b, h2h_w, h2h_b,
    w2h_r_w, w2h_r_b, h2h_r_w, h2h_r_b, dc_w, out_w, out_b,
):
    """Host-side sharding/layout. Returns per-core input dicts."""
    b, t_steps, v = input_seq.shape
    h = last_hidden.shape[1]
    d = last_dt.shape[1]
    cols = t_steps * BL
    v_pad = ((v + 127) // 128) * 128

    # weights (shared by all cores)
    w1cat = np.concatenate([w2h_r_w, w2h_w], axis=0)          # (4224, v)
    w1T = np.zeros((v_pad, NM * 128), np.float32)
    w1T[:v] = w1cat.T
    w1ch = np.ascontiguousarray(
        w1T.reshape(KV, 128, NM, 128).transpose(2, 1, 0, 3)
    ).astype(BF16)
    wcatT = np.ascontiguousarray(
        np.concatenate([(ALPHA * h2h_r_w).T, h2h_w.T], axis=1)
    ).astype(BF16)                                            # (h, 4224)
    dcT = np.ascontiguousarray(dc_w.T).astype(np.float32)     # (d, h)
    owT = np.zeros((h, v_pad), np.float32)
    owT[:, :v] = out_w.T
    owT = owT.astype(BF16)
    biasG = np.zeros((128, NM), np.float32)
    biasG[:, 0] = w2h_r_b + ALPHA * h2h_r_b
    biasG[:, 1:] = (w2h_b + h2h_b).reshape(32, 128).T
    ob = np.zeros(v_pad, np.float32)
    ob[:v] = out_b
    biasO = np.ascontiguousarray(ob.reshape(NVT, 128).T)
    ident = np.eye(128, dtype=BF16)

    in_maps = []
    for c in range(NCORE):
        bs = slice(c * BL, (c + 1) * BL)
        xsT = np.zeros((v_pad, cols), np.float32)
        xr = xsT[:v].reshape(v, t_steps, BL)
        xr[:, 1:, :] = input_seq[bs].transpose(2, 1, 0)[:, : t_steps - 1, :]
        xr[0, 0, :] = 1.0  # SOS one-hot
        in_maps.append(
            {
                "xsT": xsT.astype(BF16),
                "w1ch": w1ch,
                "wcatT": wcatT,
                "dcT": dcT,
                "owT": owT,
                "biasG": biasG,
                "biasO": biasO,
                "identI": ident,
                "hidT0": np.ascontiguousarray(last_hidden[bs].T).astype(BF16),
                "cellT0": np.ascontiguousarray(last_hidden[bs].T).astype(
                    np.float32
                ),
                "dtT0": np.ascontiguousarray(last_dt[bs].T).astype(np.float32),
            }
        )
    return in_maps, cols, v_pad, v


def _assemble(results, t_steps=T, v=V):
    """Stack per-core outc tensors back into the full (B, T, V) output."""
    out = np.empty((B, t_steps, v), np.float32)
    for c in range(NCORE):
        o = np.asarray(results[c]["outc"])  # (NVT, 128, cols)
        out[c * BL : (c + 1) * BL] = (
            o.reshape(NVT, 128, t_steps, BL)
            .transpose(3, 2, 0, 1)
            .reshape(BL, t_steps, NVT * 128)[:, :, :v]
        )
    return out


def kernel(**inputs):
    from concourse.bass_utils import run_bass_kernel_spmd

    input_seq = np.asarray(inputs["input_seq"], np.float32)
    b, t_steps, v = input_seq.shape
    args = {
        k: np.asarray(inputs[k], np.float32)
        for k in (
            "last_hidden", "last_dt", "w2h_w", "w2h_b", "h2h_w", "h2h_b",
            "w2h_r_w", "w2h_r_b", "h2h_r_w", "h2h_r_b", "dc_w", "out_w", "out_b",
        )
    }
    in_maps, _, v_pad, _ = _prep_inputs(input_seq, **args)
    nc = _cached_module(t_steps, v_pad, t_steps * BL, v)
    res = run_bass_kernel_spmd(nc, in_maps, core_ids=list(range(NCORE)))
    return np.ascontiguousarray(_assemble(res.results, t_steps, v))


# revision 35
# speedup vs baseline: 3.0643x; 1.0394x over previous
"""Trainium2 Bass kernel for nn_Decoder (LSTM-style decoder with r/dt side path).

Reference math (per step t, teacher forcing):
    xs_t    = SOS one-hot (t=0) or input_seq[:, t-1]
    z       = xs_t @ w2h_w.T + w2h_b + hid @ h2h_w.T + h2h_b          (B, 4H)
    gi,gf,go = sigmoid(z[:, 0:H]), sigmoid(z[:, H:2H]), sigmoid(z[:, 2H:3H])
    chat    = tanh(z[:, 3H:4H])
    gr      = sigmoid(xs_t @ w2h_r_w.T + w2h_r_b + a*(hid @ h2h_r_w.T + h2h_r_b))
    dt      = gr * dt
    cell    = gf*cell + gi*chat + dt @ dc_w.T
    hid     = go * tanh(cell)
    logits  = hid @ out_w.T + out_b

Distribution: data-parallel over batch (the sharding_hint's primary option).
Each of the 8 cores runs 8 of the 64 sequences end-to-end with replicated
weights — no collectives and no cross-core synchronization anywhere.

Per-core schedule (features on partitions, (t, b_local) on free dims):
  GEMM1  pre = w1.T @ xs   (4224, 600): column half A runs up front; half B
         is sliced into small PE bursts interleaved between scan steps so
         the Tensor engine stays busy while the scan's activation / vector
         chain runs. w1 (66MB) streams through SBUF one row-tile at a time,
         once per column half.
  scan   75 steps. All 41 per-step PSUM slices (33 z-tiles + 8 dc-tiles)
         live in ONE 2KB PSUM bank: a single identity-matmul injection
         (start=True) pending-zeroes the bank and seeds z with pre[t];
         every following matmul accumulates with start=False into its own
         disjoint slice. Gate slices are ordered so r|gi|gf finish first
         and go last, shortening the serial chain. The hidden history stays
         resident in SBUF — no DRAM round-trip in the scan.
  GEMM2  logits = ow.T @ hist: the first column half interleaves with the
         remaining scan steps, the rest runs as a short tail.
"""

import functools

import numpy as np
import ml_dtypes

B = 64
T = 75
V = 8000
H = 1024
D = 128
ALPHA = 0.5
NCORE = 8
BL = B // NCORE          # 8: per-core batch
COLS = T * BL            # 600: per-core (t, b) columns
V_PAD = 8064             # 63 * 128
KV = V_PAD // 128        # 63 K-tiles for GEMM1
KH = H // 128            # 8 K-tiles for the scan / GEMM2
NM = 33                  # GEMM1 / z output row tiles: r(1) + gates(32)
NZ = NM + KH             # 41: z tiles + dc tiles share one PSUM bank
NVT = V_PAD // 128       # 63 vocab tiles
NCH = COLS               # kept for test.py compatibility
HW1 = 344                # columns computed before the scan starts (mult of BL)
HW2 = COLS - HW1

BF16 = ml_dtypes.bfloat16
# GEMM2 vocab chunks (tile_start, n_tiles)
G2_CHUNKS = [(i, min(8, NVT - i)) for i in range(0, NVT, 8)]
PE_NS = 0.4167           # full-speed PE ns per output column


class _Filler:
    """Queue of (cost_ns, emit_fn, min_step) Tensor-engine work, drained in
    budgeted slices between scan-step fragments so the PE never idles while
    the scan's activation/vector chain runs. Entries gated by min_step are
    skipped until the scan has produced the data they read."""

    def __init__(self):
        self.q = []
        self.head = 0

    def add(self, cost, fn, min_step=0):
        self.q.append((cost, fn, min_step))

    def total(self):
        return sum(c for c, _, _ in self.q)

    def emit(self, budget, step=1 << 30, at_least=0):
        emitted = 0
        while self.head < len(self.q) and (budget > 0 or emitted < at_least):
            cost, fn, min_step = self.q[self.head]
            if step < min_step:
                break
            self.head += 1
            fn()
            budget -= cost
            emitted += 1
        return budget

    def drain(self):
        self.emit(float("inf"))


def _build_module(t_steps=T, v_pad=V_PAD, nch=NCH, vs=V):
    import concourse.mybir as mybir
    import concourse.tile as tile
    from concourse import bacc

    dt_ = mybir.dt
    f32, bf16 = dt_.float32, dt_.bfloat16
    AF = mybir.ActivationFunctionType

    cols = t_steps * BL
    nt1 = HW1 // BL

    nc = bacc.Bacc("TRN2", target_bir_lowering=False, num_devices=NCORE)

    # ---------------- I/O ----------------
    xsT = nc.dram_tensor("xsT", [v_pad, cols], bf16, kind="ExternalInput")
    w1ch = nc.dram_tensor("w1ch", [NM, 128, KV, 128], bf16, kind="ExternalInput")
    wcatT = nc.dram_tensor("wcatT", [H, NM * 128], bf16, kind="ExternalInput")
    dcT = nc.dram_tensor("dcT", [D, H], f32, kind="ExternalInput")
    owT = nc.dram_tensor("owT", [H, v_pad], bf16, kind="ExternalInput")
    biasG = nc.dram_tensor("biasG", [128, NM], f32, kind="ExternalInput")
    biasO = nc.dram_tensor("biasO", [128, NVT], f32, kind="ExternalInput")
    identI = nc.dram_tensor("identI", [128, 128], bf16, kind="ExternalInput")
    hidT0 = nc.dram_tensor("hidT0", [H, BL], bf16, kind="ExternalInput")
    cellT0 = nc.dram_tensor("cellT0", [H, BL], f32, kind="ExternalInput")
    dtT0 = nc.dram_tensor("dtT0", [D, BL], f32, kind="ExternalInput")
    outc = nc.dram_tensor("outc", [NVT, 128, cols], bf16, kind="ExternalOutput")

    with tile.TileContext(nc) as tc:
        import contextlib

        with contextlib.ExitStack() as ctx:
            cpool = ctx.enter_context(tc.tile_pool(name="const", bufs=1))
            spool = ctx.enter_context(tc.tile_pool(name="state", bufs=1))

            # resident constants / accumulators (tiles allocated up front;
            # their DMAs are deferred so GEMM1's xs/w1 loads go first and the
            # first matmul isn't stuck behind ~50us of constant transfers)
            pre = cpool.tile([128, t_steps, NM, BL], bf16)       # 38.7KB/part
            dc_sb = cpool.tile([128, H], f32)
            bg_sb = cpool.tile([128, NM], f32)
            bo_sb = cpool.tile([128, NVT], f32)
            id_sb = cpool.tile([128, 128], bf16)
            hid0_sb = spool.tile([128, KH, BL], bf16)
            cell_sb = spool.tile([128, KH, BL], f32)
            dt_sb = spool.tile([128, BL], f32)
            wcat_sb = cpool.tile([128, KH, NM * 128], bf16)      # 66KB/part
            hist = cpool.tile([128, KH, cols], bf16)             # 9.4KB/part

            def dma_const():
                yield lambda: nc.sync.dma_start(
                    hid0_sb[:], hidT0.ap().rearrange("(k p) n -> p k n", p=128)
                )
                yield lambda: nc.sync.dma_start(
                    cell_sb[:], cellT0.ap().rearrange("(k p) n -> p k n", p=128)
                )
                yield lambda: nc.sync.dma_start(dt_sb[:], dtT0.ap())
                yield lambda: nc.sync.dma_start(id_sb[:], identI.ap())
                # wcat (8.4MB) in per-k slices that slot into w1 DMA gaps
                for kk in range(KH):
                    yield lambda kk=kk: nc.sync.dma_start(
                        wcat_sb[:, kk, :],
                        wcatT.ap()[kk * 128 : (kk + 1) * 128, :],
                    )
                yield lambda: nc.sync.dma_start(dc_sb[:], dcT.ap())
                yield lambda: nc.sync.dma_start(bo_sb[:], biasO.ap())

            const_dmas = dma_const()
            wpool = ctx.enter_context(tc.tile_pool(name="work", bufs=2))
            zpool = ctx.enter_context(
                tc.tile_pool(name="zp", bufs=2, space="PSUM")
            )

            evict_flip = {"v": 0}

            def evict(dst, src, bias):
                # alternate the psum->sbuf bias-add between DVE and Act so
                # neither engine's scan-chain work queues behind evictions
                evict_flip["v"] ^= 1
                if evict_flip["v"]:
                    nc.vector.tensor_scalar_add(dst, src, bias)
                else:
                    nc.scalar.activation(dst, src, AF.Identity, bias=bias)

            def scan_a(t):
                # one bank: [r|gi|gf|go|chat](33) + dc(8), all f32 x BL
                pz = zpool.tile([128, NZ, BL], f32, tag="z", name=f"z{t}")
                # identity injection seeds z with pre[t] and pending-zeroes
                # the whole bank (incl. the dc slices)
                nc.tensor.matmul(
                    pz[:, 0:NM, :], id_sb[:], pre[:, t, :, :],
                    start=True, stop=False,
                )

                def rhs(k):
                    return (
                        hid0_sb[:, k, :]
                        if t == 0
                        else hist[:, k, (t - 1) * BL : t * BL]
                    )

                # m-outer so early slices complete first: r|gi|gf feed the dt
                # and cell chains, chat feeds gi*chat, go is needed last.
                for m in list(range(17)) + list(range(25, NM)) + list(range(17, 25)):
                    for k in range(KH):
                        nc.tensor.matmul(
                            pz[:, m, :],
                            wcat_sb[:, k, m * 128 : (m + 1) * 128],
                            rhs(k),
                            start=False,
                            stop=False,
                        )
                sg = wpool.tile([128, 25, BL], f32, tag="sg")
                th = wpool.tile([128, KH, BL], f32, tag="th")
                nc.scalar.activation(sg[:, 0:17, :], pz[:, 0:17, :], AF.Sigmoid)
                nc.scalar.activation(th[:], pz[:, 25:NM, :], AF.Tanh)
                nc.scalar.activation(sg[:, 17:25, :], pz[:, 17:25, :], AF.Sigmoid)
                nc.vector.tensor_mul(dt_sb[:], sg[:, 0, :], dt_sb[:])
                return pz, sg, th

            def scan_b(t, pz, sg, th):
                # dc = dc_w @ dt accumulated into the bank (f32 operands)
                for hm in range(KH):
                    nc.tensor.matmul(
                        pz[:, NM + hm, :],
                        dc_sb[:, hm * 128 : (hm + 1) * 128],
                        dt_sb[:],
                        start=False,
                        stop=(hm == KH - 1),
                    )
                # cell = gf*cell + gi*chat + dc
                tmp = wpool.tile([128, KH, BL], f32, tag="tmp")
                nc.vector.tensor_mul(cell_sb[:], sg[:, 9:17, :], cell_sb[:])
                nc.vector.tensor_mul(tmp[:], sg[:, 1:9, :], th[:])
                nc.vector.tensor_add(cell_sb[:], cell_sb[:], tmp[:])
                nc.vector.tensor_add(cell_sb[:], cell_sb[:], pz[:, NM:NZ, :])
                # hid = go * tanh(cell), written straight into the history
                thc = wpool.tile([128, KH, BL], f32, tag="thc")
                nc.scalar.activation(thc[:], cell_sb[:], AF.Tanh)
                nc.vector.tensor_mul(
                    hist[:, :, t * BL : (t + 1) * BL], sg[:, 17:25, :], thc[:]
                )

            # ---- GEMM1 phase A + interleaved phase B ----
            with contextlib.ExitStack() as c1:
                xpool = c1.enter_context(tc.tile_pool(name="xs", bufs=1))
                w1pool = c1.enter_context(tc.tile_pool(name="w1", bufs=2))
                gpsum = c1.enter_context(
                    tc.tile_pool(name="g1p", bufs=2, space="PSUM")
                )

                w1_tiles = {}

                def w1_dma(u):
                    if u in w1_tiles or u >= 2 * NM:
                        return
                    w1sb = w1pool.tile([128, KV, 128], bf16, tag="w1")
                    nc.sync.dma_start(w1sb[:], w1ch.ap()[u % NM])
                    w1_tiles[u] = w1sb

                # xs half A in k-slices so unit 0 starts as early as possible
                xs_a = xpool.tile([128, KV, HW1], bf16, tag="xs")
                nc.sync.dma_start(
                    xs_a[:, 0:12, :],
                    xsT.ap()[: 12 * 128, 0:HW1].rearrange(
                        "(k p) n -> p k n", p=128
                    ),
                )
                w1_dma(0)
                for k0, k1 in ((12, 28), (28, 45), (45, KV)):
                    nc.sync.dma_start(
                        xs_a[:, k0:k1, :],
                        xsT.ap()[k0 * 128 : k1 * 128, 0:HW1].rearrange(
                            "(k p) n -> p k n", p=128
                        ),
                    )
                nc.sync.dma_start(bg_sb[:], biasG.ap())
                for u in range(NM):
                    w1_dma(u)
                    w1_dma(u + 1)
                    pg = gpsum.tile([128, HW1], f32, tag="pg", name=f"pga{u}")
                    for k in range(KV):
                        nc.tensor.matmul(
                            pg[:],
                            w1_tiles[u][:, k, :],
                            xs_a[:, k, :],
                            start=(k == 0),
                            stop=(k == KV - 1),
                        )
                    evict(pre[:, 0:nt1, u, :], pg[:], bg_sb[:, u : u + 1])
                    w1_tiles.pop(u, None)
                    # slot one deferred constant DMA behind each unit so they
                    # fill w1-stream gaps without delaying the w1 prefetches
                    if u >= 1:
                        fn = next(const_dmas, None)
                        if fn is not None:
                            fn()

                # phase B input (reuses the xs buffer; WAR-serialized by
                # Tile). Loaded in k-slices so the first B units start as
                # soon as their k-range has landed.
                xs_b = xpool.tile([128, KV, HW1], bf16, tag="xs")
                for k0, nk in ((0, 21), (21, 21), (42, 21)):
                    nc.sync.dma_start(
                        xs_b[:, k0 : k0 + nk, 0:HW2],
                        xsT.ap()[k0 * 128 : (k0 + nk) * 128, HW1:cols].rearrange(
                            "(k p) n -> p k n", p=128
                        ),
                    )

                # queue phase-B units as ~2.6us k-slices
                g1fill = _Filler()
                KSPLIT = [(0, 21), (21, 21), (42, 21)]

                def g1b_slice(m, k0, nk):
                    def emit():
                        if k0 == 0:
                            w1_dma(NM + m)
                            w1_dma(NM + m + 1)
                        pg = g1fill.pg if k0 else gpsum.tile(
                            [128, HW1], f32, tag="pg", name=f"pgb{m}"
                        )
                        g1fill.pg = pg
                        for k in range(k0, k0 + nk):
                            nc.tensor.matmul(
                                pg[:, 0:HW2],
                                w1_tiles[NM + m][:, k, :],
                                xs_b[:, k, 0:HW2],
                                start=(k == 0),
                                stop=(k == KV - 1),
                            )
                        if k0 + nk == KV:
                            evict(
                                pre[:, nt1 : cols // BL, m, :],
                                pg[:, 0:HW2],
                                bg_sb[:, m : m + 1],
                            )
                            w1_tiles.pop(NM + m, None)

                    return emit

                for m in range(NM):
                    for k0, nk in KSPLIT:
                        g1fill.add(nk * HW2 * PE_NS, g1b_slice(m, k0, nk))

                quota = g1fill.total() / max(1, nt1 - 2)
                deficit = 0.0
                for t in range(min(nt1, t_steps)):
                    deficit += quota
                    pz, sg, th = scan_a(t)
                    deficit -= 1800.0 - g1fill.emit(1800.0, at_least=1)
                    scan_b(t, pz, sg, th)
                    deficit = g1fill.emit(deficit)
                g1fill.drain()

            # ---- GEMM2 pass A + scan steps nt1..T-1, then the tail ----
            with contextlib.ExitStack() as c2:
                opool = c2.enter_context(tc.tile_pool(name="ow", bufs=2))
                ospool = c2.enter_context(tc.tile_pool(name="os", bufs=2))
                opsum = c2.enter_context(
                    tc.tile_pool(name="g2p", bufs=2, space="PSUM")
                )

                ow_tiles = {}

                def ow_dma(ci):
                    if ci in ow_tiles or not (0 <= ci < len(G2_CHUNKS)):
                        return
                    v0, nt = G2_CHUNKS[ci]
                    ow_sb = opool.tile([128, KH, 8 * 128], bf16, tag="ow")
                    nc.sync.dma_start(
                        ow_sb[:, :, 0 : nt * 128],
                        owT.ap()[:, v0 * 128 : (v0 + nt) * 128].rearrange(
                            "(k p) m -> p k m", p=128
                        ),
                    )
                    ow_tiles[ci] = ow_sb

                osb_cur = {}

                def g2_unit(ci, mi, h0, hw, last, prefetch):
                    def emit():
                        if mi == 0:
                            ow_dma(ci)
                            osb_cur["t"] = ospool.tile(
                                [128, 8, HW1], bf16, tag="osb",
                                name=f"osb{h0}_{ci}",
                            )
                        if mi == 2:
                            ow_dma(prefetch)
                        v0, nt = G2_CHUNKS[ci]
                        m = v0 + mi
                        po = opsum.tile(
                            [128, HW1], f32, tag="po", name=f"po{h0}_{m}"
                        )
                        for k in range(KH):
                            nc.tensor.matmul(
                                po[:, 0:hw],
                                ow_tiles[ci][:, k, mi * 128 : (mi + 1) * 128],
                                hist[:, k, h0 : h0 + hw],
                                start=(k == 0),
                                stop=(k == KH - 1),
                            )
                        osb = osb_cur["t"]
                        evict(
                            osb[:, mi, 0:hw], po[:, 0:hw], bo_sb[:, m : m + 1]
                        )
                        if mi == nt - 1:
                            # one batched DMA for the whole vocab chunk
                            nc.sync.dma_start(
                                outc.ap()[v0 : v0 + nt][
                                    :, :, h0 : h0 + hw
                                ].rearrange("m p n -> p m n"),
                                osb[:, 0:nt, 0:hw],
                            )
                        if last:
                            ow_tiles.pop(ci, None)

                    return emit

                # Two column passes over the vocab: [0:HW1] interleaves with
                # the remaining scan steps, [HW1:] is the tail. Chunk order
                # alternates per pass so the chunk left resident at the pass
                # boundary is reused without a re-DMA.
                g2fill = _Filler()
                nch = len(G2_CHUNKS)

                def add_pass(order, h0, hw, min_step, filler=None, keep_last=False):
                    for j, ci in enumerate(order):
                        v0, nt = G2_CHUNKS[ci]
                        is_last = j + 1 == len(order)
                        nxt = -1 if is_last else order[j + 1]
                        for mi in range(nt):
                            u = g2_unit(
                                ci, mi, h0, hw,
                                last=(mi == nt - 1 and not (is_last and keep_last)),
                                prefetch=nxt,
                            )
                            if filler is None:
                                u()
                            else:
                                filler.add(KH * hw * PE_NS, u, min_step)

                fwd = list(range(nch))
                rev = list(reversed(fwd))
                add_pass(fwd, 0, HW1, nt1, g2fill, keep_last=True)

                quota = g2fill.total() / max(1, t_steps - nt1 - 2)
                deficit = 0.0
                for t in range(nt1, t_steps):
                    deficit += quota
                    pz, sg, th = scan_a(t)
                    deficit -= 1800.0 - g2fill.emit(1800.0, t, at_least=1)
                    scan_b(t, pz, sg, th)
                    deficit = g2fill.emit(deficit, t)
                g2fill.drain()

                # tail: remaining columns, reusing the resident last chunk
                add_pass(rev, HW1, HW2, 0, None)

    nc.finalize()
    return nc


@functools.lru_cache(maxsize=2)
def _cached_module(t_steps=T, v_pad=V_PAD, nch=NCH, vs=V):
    return _build_module(t_steps, v_pad, nch, vs)


def _prep_inputs(
    input_seq, last_hidden, last_dt, w2h_w, w2h_b, h2h_w, h2h_b,
    w2h_r_w, w2h_r_b, h2h_r_w, h2h_r_b, dc_w, out_w, out_b,
):
    """Host-side sharding/layout. Returns per-core input dicts."""
    b, t_steps, v = input_seq.shape
    h = last_hidden.shape[1]
    d = last_dt.shape[1]
    cols = t_steps * BL
    v_pad = ((v + 127) // 128) * 128

    # weights (shared by all cores)
    w1cat = np.concatenate([w2h_r_w, w2h_w], axis=0)          # (4224, v)
    w1T = np.zeros((v_pad, NM * 128), np.float32)
    w1T[:v] = w1cat.T
    w1ch = np.ascontiguousarray(
        w1T.reshape(KV, 128, NM, 128).transpose(2, 1, 0, 3)
    ).astype(BF16)
    wcatT = np.ascontiguousarray(
        np.concatenate([(ALPHA * h2h_r_w).T, h2h_w.T], axis=1)
    ).astype(BF16)                                            # (h, 4224)
    dcT = np.ascontiguousarray(dc_w.T).astype(np.float32)     # (d, h)
    owT = np.zeros((h, v_pad), np.float32)
    owT[:, :v] = out_w.T
    owT = owT.astype(BF16)
    biasG = np.zeros((128, NM), np.float32)
    biasG[:, 0] = w2h_r_b + ALPHA * h2h_r_b
    biasG[:, 1:] = (w2h_b + h2h_b).reshape(32, 128).T
    ob = np.zeros(v_pad, np.float32)
    ob[:v] = out_b
    biasO = np.ascontiguousarray(ob.reshape(NVT, 128).T)
    ident = np.eye(128, dtype=BF16)

    in_maps = []
    for c in range(NCORE):
        bs = slice(c * BL, (c + 1) * BL)
        xsT = np.zeros((v_pad, cols), np.float32)
        xr = xsT[:v].reshape(v, t_steps, BL)
        xr[:, 1:, :] = input_seq[bs].transpose(2, 1, 0)[:, : t_steps - 1, :]
        xr[0, 0, :] = 1.0  # SOS one-hot
        in_maps.append(
            {
                "xsT": xsT.astype(BF16),
                "w1ch": w1ch,
                "wcatT": wcatT,
                "dcT": dcT,
                "owT": owT,
                "biasG": biasG,
                "biasO": biasO,
                "identI": ident,
                "hidT0": np.ascontiguousarray(last_hidden[bs].T).astype(BF16),
                "cellT0": np.ascontiguousarray(last_hidden[bs].T).astype(
                    np.float32
                ),
                "dtT0": np.ascontiguousarray(last_dt[bs].T).astype(np.float32),
            }
        )
    return in_maps, cols, v_pad, v


def _assemble(results, t_steps=T, v=V):
    """Stack per-core outc tensors back into the full (B, T, V) output."""
    out = np.empty((B, t_steps, v), np.float32)
    for c in range(NCORE):
        o = np.asarray(results[c]["outc"])  # (NVT, 128, cols)
        out[c * BL : (c + 1) * BL] = (
            o.reshape(NVT, 128, t_steps, BL)
            .transpose(3, 2, 0, 1)
            .reshape(BL, t_steps, NVT * 128)[:, :, :v]
        )
    return out


def kernel(**inputs):
    from concourse.bass_utils import run_bass_kernel_spmd

    input_seq = np.asarray(inputs["input_seq"], np.float32)
    b, t_steps, v = input_seq.shape
    args = {
        k: np.asarray(inputs[k], np.float32)
        for k in (
            "last_hidden", "last_dt", "w2h_w", "w2h_b", "h2h_w", "h2h_b",
            "w2h_r_w", "w2h_r_b", "h2h_r_w", "h2h_r_b", "dc_w", "out_w", "out_b",
        )
    }
    in_maps, _, v_pad, _ = _prep_inputs(input_seq, **args)
    nc = _cached_module(t_steps, v_pad, t_steps * BL, v)
    res = run_bass_kernel_spmd(nc, in_maps, core_ids=list(range(NCORE)))
    return np.ascontiguousarray(_assemble(res.results, t_steps, v))


# revision 37
# speedup vs baseline: 3.0658x; 1.0005x over previous
"""Trainium2 Bass kernel for nn_Decoder (LSTM-style decoder with r/dt side path).

Reference math (per step t, teacher forcing):
    xs_t    = SOS one-hot (t=0) or input_seq[:, t-1]
    z       = xs_t @ w2h_w.T + w2h_b + hid @ h2h_w.T + h2h_b          (B, 4H)
    gi,gf,go = sigmoid(z[:, 0:H]), sigmoid(z[:, H:2H]), sigmoid(z[:, 2H:3H])
    chat    = tanh(z[:, 3H:4H])
    gr      = sigmoid(xs_t @ w2h_r_w.T + w2h_r_b + a*(hid @ h2h_r_w.T + h2h_r_b))
    dt      = gr * dt
    cell    = gf*cell + gi*chat + dt @ dc_w.T
    hid     = go * tanh(cell)
    logits  = hid @ out_w.T + out_b

Distribution: data-parallel over batch (the sharding_hint's primary option).
Each of the 8 cores runs 8 of the 64 sequences end-to-end with replicated
weights — no collectives and no cross-core synchronization anywhere.

Per-core schedule (features on partitions, (t, b_local) on free dims):
  GEMM1  pre = w1.T @ xs   (4224, 600): columns 0:HW1 run up front; the
         remaining columns are sliced into ~2us PE bursts interleaved
         between scan steps so the Tensor engine stays busy while the
         scan's activation / vector chain runs. w1 (66MB) streams through
         SBUF one row-tile at a time, once per column block.
  scan   75 steps. All 41 per-step PSUM slices (33 z-tiles + 8 dc-tiles)
         live in ONE 2KB PSUM bank: a single identity-matmul injection
         (start=True) pending-zeroes the bank and seeds z with pre[t];
         every following matmul accumulates with start=False into its own
         disjoint slice, and only the bank's final matmul carries
         stop=True. Gate slices are ordered so r|gi|gf finish first and go
         last, shortening the serial chain. The hidden history stays
         resident in SBUF — no DRAM round-trip in the scan.
  GEMM2  logits = ow.T @ hist: columns 0:HW1 interleave with the remaining
         scan steps (vocab streamed in 8-tile chunks, one batched output
         DMA per chunk), the rest runs as a short tail.

Measured (TimelineSim of the compiled stream): ~812us vs the 2489us
AllGather-per-step baseline; hardware rel err vs the fp64 reference 0.0037.
"""

import functools

import numpy as np
import ml_dtypes

B = 64
T = 75
V = 8000
H = 1024
D = 128
ALPHA = 0.5
NCORE = 8
BL = B // NCORE          # 8: per-core batch
COLS = T * BL            # 600: per-core (t, b) columns
V_PAD = 8064             # 63 * 128
KV = V_PAD // 128        # 63 K-tiles for GEMM1
KH = H // 128            # 8 K-tiles for the scan / GEMM2
NM = 33                  # GEMM1 / z output row tiles: r(1) + gates(32)
NZ = NM + KH             # 41: z tiles + dc tiles share one PSUM bank
NVT = V_PAD // 128       # 63 vocab tiles
NCH = COLS               # kept for test.py compatibility
HW1 = 344                # columns computed before the scan starts (mult of BL)
HW2 = COLS - HW1

BF16 = ml_dtypes.bfloat16
# GEMM2 vocab chunks (tile_start, n_tiles)
G2_CHUNKS = [(i, min(8, NVT - i)) for i in range(0, NVT, 8)]
PE_NS = 0.4167           # full-speed PE ns per output column


class _Filler:
    """Queue of (cost_ns, emit_fn, min_step) Tensor-engine work, drained in
    budgeted slices between scan-step fragments so the PE never idles while
    the scan's activation/vector chain runs. Entries gated by min_step are
    skipped until the scan has produced the data they read."""

    def __init__(self):
        self.q = []
        self.head = 0

    def add(self, cost, fn, min_step=0):
        self.q.append((cost, fn, min_step))

    def total(self):
        return sum(c for c, _, _ in self.q)

    def emit(self, budget, step=1 << 30, at_least=0):
        emitted = 0
        while self.head < len(self.q) and (budget > 0 or emitted < at_least):
            cost, fn, min_step = self.q[self.head]
            if step < min_step:
                break
            self.head += 1
            fn()
            budget -= cost
            emitted += 1
        return budget

    def drain(self):
        self.emit(float("inf"))


def _build_module(t_steps=T, v_pad=V_PAD, nch=NCH, vs=V):
    import concourse.mybir as mybir
    import concourse.tile as tile
    from concourse import bacc

    dt_ = mybir.dt
    f32, bf16 = dt_.float32, dt_.bfloat16
    AF = mybir.ActivationFunctionType

    cols = t_steps * BL
    nt1 = HW1 // BL

    nc = bacc.Bacc("TRN2", target_bir_lowering=False, num_devices=NCORE)

    # ---------------- I/O ----------------
    xsT = nc.dram_tensor("xsT", [v_pad, cols], bf16, kind="ExternalInput")
    w1ch = nc.dram_tensor("w1ch", [NM, 128, KV, 128], bf16, kind="ExternalInput")
    wcatT = nc.dram_tensor("wcatT", [H, NM * 128], bf16, kind="ExternalInput")
    dcT = nc.dram_tensor("dcT", [D, H], f32, kind="ExternalInput")
    owT = nc.dram_tensor("owT", [H, v_pad], bf16, kind="ExternalInput")
    biasG = nc.dram_tensor("biasG", [128, NM], f32, kind="ExternalInput")
    biasO = nc.dram_tensor("biasO", [128, NVT], f32, kind="ExternalInput")
    identI = nc.dram_tensor("identI", [128, 128], bf16, kind="ExternalInput")
    hidT0 = nc.dram_tensor("hidT0", [H, BL], bf16, kind="ExternalInput")
    cellT0 = nc.dram_tensor("cellT0", [H, BL], f32, kind="ExternalInput")
    dtT0 = nc.dram_tensor("dtT0", [D, BL], f32, kind="ExternalInput")
    outc = nc.dram_tensor("outc", [NVT, 128, cols], bf16, kind="ExternalOutput")

    with tile.TileContext(nc) as tc:
        import contextlib

        with contextlib.ExitStack() as ctx:
            cpool = ctx.enter_context(tc.tile_pool(name="const", bufs=1))
            spool = ctx.enter_context(tc.tile_pool(name="state", bufs=1))

            # resident constants / accumulators (tiles allocated up front;
            # their DMAs are deferred so GEMM1's xs/w1 loads go first and the
            # first matmul isn't stuck behind ~50us of constant transfers)
            pre = cpool.tile([128, t_steps, NM, BL], bf16)       # 38.7KB/part
            dc_sb = cpool.tile([128, H], f32)
            bg_sb = cpool.tile([128, NM], f32)
            bo_sb = cpool.tile([128, NVT], f32)
            id_sb = cpool.tile([128, 128], bf16)
            hid0_sb = spool.tile([128, KH, BL], bf16)
            cell_sb = spool.tile([128, KH, BL], f32)
            dt_sb = spool.tile([128, BL], f32)
            wcat_sb = cpool.tile([128, KH, NM * 128], bf16)      # 66KB/part
            hist = cpool.tile([128, KH, cols], bf16)             # 9.4KB/part

            def dma_const():
                yield lambda: nc.sync.dma_start(
                    hid0_sb[:], hidT0.ap().rearrange("(k p) n -> p k n", p=128)
                )
                yield lambda: nc.sync.dma_start(
                    cell_sb[:], cellT0.ap().rearrange("(k p) n -> p k n", p=128)
                )
                yield lambda: nc.sync.dma_start(dt_sb[:], dtT0.ap())
                yield lambda: nc.sync.dma_start(id_sb[:], identI.ap())
                # wcat (8.4MB) in per-k slices that slot into w1 DMA gaps
                for kk in range(KH):
                    yield lambda kk=kk: nc.sync.dma_start(
                        wcat_sb[:, kk, :],
                        wcatT.ap()[kk * 128 : (kk + 1) * 128, :],
                    )
                yield lambda: nc.sync.dma_start(dc_sb[:], dcT.ap())
                yield lambda: nc.sync.dma_start(bo_sb[:], biasO.ap())

            const_dmas = dma_const()
            wpool = ctx.enter_context(tc.tile_pool(name="work", bufs=2))
            zpool = ctx.enter_context(
                tc.tile_pool(name="zp", bufs=3, space="PSUM")
            )

            evict_flip = {"v": 0}

            def evict(dst, src, bias):
                # alternate the psum->sbuf bias-add between DVE and Act so
                # neither engine's scan-chain work queues behind evictions
                evict_flip["v"] ^= 1
                if evict_flip["v"]:
                    nc.vector.tensor_scalar_add(dst, src, bias)
                else:
                    nc.scalar.activation(dst, src, AF.Identity, bias=bias)

            def scan_a(t):
                # one bank: [r|gi|gf|go|chat](33) + dc(8), all f32 x BL
                pz = zpool.tile([128, NZ, BL], f32, tag="z", name=f"z{t}")
                # identity injection seeds z with pre[t] and pending-zeroes
                # the whole bank (incl. the dc slices)
                nc.tensor.matmul(
                    pz[:, 0:NM, :], id_sb[:], pre[:, t, :, :],
                    start=True, stop=False,
                )

                def rhs(k):
                    return (
                        hid0_sb[:, k, :]
                        if t == 0
                        else hist[:, k, (t - 1) * BL : t * BL]
                    )

                # m-outer so early slices complete first: r|gi|gf feed the dt
                # and cell chains, chat feeds gi*chat, go is needed last.
                for m in list(range(17)) + list(range(25, NM)) + list(range(17, 25)):
                    for k in range(KH):
                        nc.tensor.matmul(
                            pz[:, m, :],
                            wcat_sb[:, k, m * 128 : (m + 1) * 128],
                            rhs(k),
                            start=False,
                            stop=False,
                        )
                sg = wpool.tile([128, 25, BL], f32, tag="sg")
                th = wpool.tile([128, KH, BL], f32, tag="th")
                nc.scalar.activation(sg[:, 0:17, :], pz[:, 0:17, :], AF.Sigmoid)
                nc.scalar.activation(th[:], pz[:, 25:NM, :], AF.Tanh)
                nc.scalar.activation(sg[:, 17:25, :], pz[:, 17:25, :], AF.Sigmoid)
                nc.vector.tensor_mul(dt_sb[:], sg[:, 0, :], dt_sb[:])
                return pz, sg, th

            def scan_b(t, pz, sg, th):
                # dc = dc_w @ dt accumulated into the bank (f32 operands)
                for hm in range(KH):
                    nc.tensor.matmul(
                        pz[:, NM + hm, :],
                        dc_sb[:, hm * 128 : (hm + 1) * 128],
                        dt_sb[:],
                        start=False,
                        stop=(hm == KH - 1),
                    )
                # cell = gf*cell + gi*chat + dc
                tmp = wpool.tile([128, KH, BL], f32, tag="tmp")
                nc.vector.tensor_mul(cell_sb[:], sg[:, 9:17, :], cell_sb[:])
                nc.vector.tensor_mul(tmp[:], sg[:, 1:9, :], th[:])
                nc.vector.tensor_add(cell_sb[:], cell_sb[:], tmp[:])
                nc.vector.tensor_add(cell_sb[:], cell_sb[:], pz[:, NM:NZ, :])
                # hid = go * tanh(cell), written straight into the history
                thc = wpool.tile([128, KH, BL], f32, tag="thc")
                nc.scalar.activation(thc[:], cell_sb[:], AF.Tanh)
                nc.vector.tensor_mul(
                    hist[:, :, t * BL : (t + 1) * BL], sg[:, 17:25, :], thc[:]
                )

            # ---- GEMM1 phase A + interleaved phase B ----
            with contextlib.ExitStack() as c1:
                xpool = c1.enter_context(tc.tile_pool(name="xs", bufs=1))
                w1pool = c1.enter_context(tc.tile_pool(name="w1", bufs=2))
                gpsum = c1.enter_context(
                    tc.tile_pool(name="g1p", bufs=2, space="PSUM")
                )

                w1_tiles = {}

                def w1_dma(u):
                    if u in w1_tiles or u >= 2 * NM:
                        return
                    w1sb = w1pool.tile([128, KV, 128], bf16, tag="w1")
                    nc.sync.dma_start(w1sb[:], w1ch.ap()[u % NM])
                    w1_tiles[u] = w1sb

                # xs half A in k-slices so unit 0 starts as early as possible
                xs_a = xpool.tile([128, KV, HW1], bf16, tag="xs")
                w1_dma(0)
                nc.sync.dma_start(
                    xs_a[:, 0:12, :],
                    xsT.ap()[: 12 * 128, 0:HW1].rearrange(
                        "(k p) n -> p k n", p=128
                    ),
                )
                for k0, k1 in ((12, 28), (28, 45), (45, KV)):
                    nc.sync.dma_start(
                        xs_a[:, k0:k1, :],
                        xsT.ap()[k0 * 128 : k1 * 128, 0:HW1].rearrange(
                            "(k p) n -> p k n", p=128
                        ),
                    )
                nc.sync.dma_start(bg_sb[:], biasG.ap())
                for u in range(NM):
                    w1_dma(u)
                    w1_dma(u + 1)
                    pg = gpsum.tile([128, HW1], f32, tag="pg", name=f"pga{u}")
                    for k in range(KV):
                        nc.tensor.matmul(
                            pg[:],
                            w1_tiles[u][:, k, :],
                            xs_a[:, k, :],
                            start=(k == 0),
                            stop=(k == KV - 1),
                        )
                    evict(pre[:, 0:nt1, u, :], pg[:], bg_sb[:, u : u + 1])
                    w1_tiles.pop(u, None)
                    # slot one deferred constant DMA behind each unit so they
                    # fill w1-stream gaps without delaying the w1 prefetches
                    if u >= 1:
                        fn = next(const_dmas, None)
                        if fn is not None:
                            fn()

                # phase B input (reuses the xs buffer; WAR-serialized by
                # Tile). Loaded in k-slices so the first B units start as
                # soon as their k-range has landed.
                xs_b = xpool.tile([128, KV, HW1], bf16, tag="xs")
                for k0, nk in ((0, 21), (21, 21), (42, 21)):
                    nc.sync.dma_start(
                        xs_b[:, k0 : k0 + nk, 0:HW2],
                        xsT.ap()[k0 * 128 : (k0 + nk) * 128, HW1:cols].rearrange(
                            "(k p) n -> p k n", p=128
                        ),
                    )

                # queue phase-B units as ~2.6us k-slices
                g1fill = _Filler()
                KSPLIT = [(0, 21), (21, 21), (42, 21)]

                def g1b_slice(m, k0, nk):
                    def emit():
                        if k0 == 0:
                            w1_dma(NM + m)
                            w1_dma(NM + m + 1)
                        pg = g1fill.pg if k0 else gpsum.tile(
                            [128, HW1], f32, tag="pg", name=f"pgb{m}"
                        )
                        g1fill.pg = pg
                        for k in range(k0, k0 + nk):
                            nc.tensor.matmul(
                                pg[:, 0:HW2],
                                w1_tiles[NM + m][:, k, :],
                                xs_b[:, k, 0:HW2],
                                start=(k == 0),
                                stop=(k == KV - 1),
                            )
                        if k0 + nk == KV:
                            evict(
                                pre[:, nt1 : cols // BL, m, :],
                                pg[:, 0:HW2],
                                bg_sb[:, m : m + 1],
                            )
                            w1_tiles.pop(NM + m, None)

                    return emit

                for m in range(NM):
                    for k0, nk in KSPLIT:
                        g1fill.add(nk * HW2 * PE_NS, g1b_slice(m, k0, nk))

                quota = g1fill.total() / max(1, nt1 - 2)
                deficit = 0.0
                for t in range(min(nt1, t_steps)):
                    deficit += quota
                    pz, sg, th = scan_a(t)
                    deficit -= 1800.0 - g1fill.emit(1800.0, at_least=1)
                    scan_b(t, pz, sg, th)
                    deficit = g1fill.emit(deficit)
                g1fill.drain()

            # ---- GEMM2 pass A + scan steps nt1..T-1, then the tail ----
            with contextlib.ExitStack() as c2:
                opool = c2.enter_context(tc.tile_pool(name="ow", bufs=2))
                ospool = c2.enter_context(tc.tile_pool(name="os", bufs=2))
                opsum = c2.enter_context(
                    tc.tile_pool(name="g2p", bufs=2, space="PSUM")
                )

                ow_tiles = {}

                def ow_dma(ci):
                    if ci in ow_tiles or not (0 <= ci < len(G2_CHUNKS)):
                        return
                    v0, nt = G2_CHUNKS[ci]
                    ow_sb = opool.tile([128, KH, 8 * 128], bf16, tag="ow")
                    nc.sync.dma_start(
                        ow_sb[:, :, 0 : nt * 128],
                        owT.ap()[:, v0 * 128 : (v0 + nt) * 128].rearrange(
                            "(k p) m -> p k m", p=128
                        ),
                    )
                    ow_tiles[ci] = ow_sb

                osb_cur = {}

                def g2_unit(ci, mi, h0, hw, last, prefetch):
                    def emit():
                        if mi == 0:
                            ow_dma(ci)
                            osb_cur["t"] = ospool.tile(
                                [128, 8, HW1], bf16, tag="osb",
                                name=f"osb{h0}_{ci}",
                            )
                        if mi == 2:
                            ow_dma(prefetch)
                        v0, nt = G2_CHUNKS[ci]
                        m = v0 + mi
                        po = opsum.tile(
                            [128, HW1], f32, tag="po", name=f"po{h0}_{m}"
                        )
                        for k in range(KH):
                            nc.tensor.matmul(
                                po[:, 0:hw],
                                ow_tiles[ci][:, k, mi * 128 : (mi + 1) * 128],
                                hist[:, k, h0 : h0 + hw],
                                start=(k == 0),
                                stop=(k == KH - 1),
                            )
                        osb = osb_cur["t"]
                        evict(
                            osb[:, mi, 0:hw], po[:, 0:hw], bo_sb[:, m : m + 1]
                        )
                        if mi == nt - 1:
                            # one batched DMA for the whole vocab chunk
                            nc.sync.dma_start(
                                outc.ap()[v0 : v0 + nt][
                                    :, :, h0 : h0 + hw
                                ].rearrange("m p n -> p m n"),
                                osb[:, 0:nt, 0:hw],
                            )
                        if last:
                            ow_tiles.pop(ci, None)

                    return emit

                # Two column passes over the vocab: [0:HW1] interleaves with
                # the remaining scan steps, [HW1:] is the tail. Chunk order
                # alternates per pass so the chunk left resident at the pass
                # boundary is reused without a re-DMA.
                g2fill = _Filler()
                nch = len(G2_CHUNKS)

                def add_pass(order, h0, hw, min_step, filler=None, keep_last=False):
                    for j, ci in enumerate(order):
                        v0, nt = G2_CHUNKS[ci]
                        is_last = j + 1 == len(order)
                        nxt = -1 if is_last else order[j + 1]
                        for mi in range(nt):
                            u = g2_unit(
                                ci, mi, h0, hw,
                                last=(mi == nt - 1 and not (is_last and keep_last)),
                                prefetch=nxt,
                            )
                            if filler is None:
                                u()
                            else:
                                filler.add(KH * hw * PE_NS, u, min_step)

                fwd = list(range(nch))
                rev = list(reversed(fwd))
                add_pass(fwd, 0, HW1, nt1, g2fill, keep_last=True)

                quota = g2fill.total() / max(1, t_steps - nt1 - 2)
                deficit = 0.0
                for t in range(nt1, t_steps):
                    deficit += quota
                    pz, sg, th = scan_a(t)
                    deficit -= 1800.0 - g2fill.emit(1800.0, t, at_least=1)
                    scan_b(t, pz, sg, th)
                    deficit = g2fill.emit(deficit, t)
                g2fill.drain()

                # tail: remaining columns, reusing the resident last chunk
                add_pass(rev, HW1, HW2, 0, None)

    nc.finalize()
    return nc


@functools.lru_cache(maxsize=2)
def _cached_module(t_steps=T, v_pad=V_PAD, nch=NCH, vs=V):
    return _build_module(t_steps, v_pad, nch, vs)


def _prep_inputs(
    input_seq, last_hidden, last_dt, w2h_w, w2h_b, h2h_w, h2h_b,
    w2h_r_w, w2h_r_b, h2h_r_w, h2h_r_b, dc_w, out_w, out_b,
):
    """Host-side sharding/layout. Returns per-core input dicts."""
    b, t_steps, v = input_seq.shape
    h = last_hidden.shape[1]
    d = last_dt.shape[1]
    cols = t_steps * BL
    v_pad = ((v + 127) // 128) * 128

    # weights (shared by all cores)
    w1cat = np.concatenate([w2h_r_w, w2h_w], axis=0)          # (4224, v)
    w1T = np.zeros((v_pad, NM * 128), np.float32)
    w1T[:v] = w1cat.T
    w1ch = np.ascontiguousarray(
        w1T.reshape(KV, 128, NM, 128).transpose(2, 1, 0, 3)
    ).astype(BF16)
    wcatT = np.ascontiguousarray(
        np.concatenate([(ALPHA * h2h_r_w).T, h2h_w.T], axis=1)
    ).astype(BF16)                                            # (h, 4224)
    dcT = np.ascontiguousarray(dc_w.T).astype(np.float32)     # (d, h)
    owT = np.zeros((h, v_pad), np.float32)
    owT[:, :v] = out_w.T
    owT = owT.astype(BF16)
    biasG = np.zeros((128, NM), np.float32)
    biasG[:, 0] = w2h_r_b + ALPHA * h2h_r_b
    biasG[:, 1:] = (w2h_b + h2h_b).reshape(32, 128).T
    ob = np.zeros(v_pad, np.float32)
    ob[:v] = out_b
    biasO = np.ascontiguousarray(ob.reshape(NVT, 128).T)
    ident = np.eye(128, dtype=BF16)

    in_maps = []
    for c in range(NCORE):
        bs = slice(c * BL, (c + 1) * BL)
        xsT = np.zeros((v_pad, cols), np.float32)
        xr = xsT[:v].reshape(v, t_steps, BL)
        xr[:, 1:, :] = input_seq[bs].transpose(2, 1, 0)[:, : t_steps - 1, :]
        xr[0, 0, :] = 1.0  # SOS one-hot
        in_maps.append(
            {
                "xsT": xsT.astype(BF16),
                "w1ch": w1ch,
                "wcatT": wcatT,
                "dcT": dcT,
                "owT": owT,
                "biasG": biasG,
                "biasO": biasO,
                "identI": ident,
                "hidT0": np.ascontiguousarray(last_hidden[bs].T).astype(BF16),
                "cellT0": np.ascontiguousarray(last_hidden[bs].T).astype(
                    np.float32
                ),
                "dtT0": np.ascontiguousarray(last_dt[bs].T).astype(np.float32),
            }
        )
    return in_maps, cols, v_pad, v


def _assemble(results, t_steps=T, v=V):
    """Stack per-core outc tensors back into the full (B, T, V) output."""
    out = np.empty((B, t_steps, v), np.float32)
    for c in range(NCORE):
        o = np.asarray(results[c]["outc"])  # (NVT, 128, cols)
        out[c * BL : (c + 1) * BL] = (
            o.reshape(NVT, 128, t_steps, BL)
            .transpose(3, 2, 0, 1)
            .reshape(BL, t_steps, NVT * 128)[:, :, :v]
        )
    return out


def kernel(**inputs):
    from concourse.bass_utils import run_bass_kernel_spmd

    input_seq = np.asarray(inputs["input_seq"], np.float32)
    b, t_steps, v = input_seq.shape
    args = {
        k: np.asarray(inputs[k], np.float32)
        for k in (
            "last_hidden", "last_dt", "w2h_w", "w2h_b", "h2h_w", "h2h_b",
            "w2h_r_w", "w2h_r_b", "h2h_r_w", "h2h_r_b", "dc_w", "out_w", "out_b",
        )
    }
    in_maps, _, v_pad, _ = _prep_inputs(input_seq, **args)
    nc = _cached_module(t_steps, v_pad, t_steps * BL, v)
    res = run_bass_kernel_spmd(nc, in_maps, core_ids=list(range(NCORE)))
    return np.ascontiguousarray(_assemble(res.results, t_steps, v))
